# revision 2
# baseline (speedup 1.0000x reference)
"""Trainium2 Bass kernel for nn_DQN_34136400069239 (DeepSets-style pooling).

Math (reference):
    h1  = relu(x @ pw1 + pb1)          [N, H]
    h2  = relu(h1 @ pw2 + pb2)         [N, H]
    phi = h2 @ pw3 + pb3               [N, F]
    fp  = sum(phi, axis=0)             [F]
    ... tiny rho MLP + concat(x_static) + tiny 3-layer MLP -> [OUT]

The third phi layer is linear, so fp = (sum_n h2[n]) @ pw3 + N * pb3 and the
device only computes S = sum_n relu(h2[n]) in R^H.  Data-parallel over rows:
8 cores x 50000 rows, host sums the 8 partial S vectors and runs the tail.

Default mode "v3" (measured 136.5 us local-slope vs 176.1 us for the staged
f16 baseline; rel err 2.5e-4), per 1000-row pair of 500-row blocks:
  - All-f16 matmuls: 4x K=64 layer-1 mms and 8x K=128 layer-2 mms, the
    latter in 2-mm start/stop accumulation groups.  The dense grouped PE
    stream keeps the HAM activity monitor un-throttled at 2.4 GHz (~6000
    warm cycles/pair = ~2.5 us, the measured bound).  fp8 DoubleRow
    variants ("v7": 3260 cyc/pair) are ALGEBRAICALLY cheaper but their
    sparser single-mm pattern leaves the PE clock-gated cold at 1.2 GHz
    (~2.8 us) - warm f16 beats cold fp8.
  - PSUM is pair-level and half-major (ps1_h/ps2_h = [128, 2(block), 512]),
    so every vector-engine op covers one h-half of both blocks with a
    uniform per-partition f32 bias vector and accum_out keeps per-channel
    sums:
      DVE: (a) h1 = max(ps1+b1, 0) -> f16, tensor_scalar(add, max) per half
      ACT: (b) relu(ps2+b2) + fused row-sum accum_out per half
Other modes kept for comparison: f16/f32r/f32r_split = the original staged
baseline (f16: 176 us); v3fp8/v5/v6/v7/v8 = fp8-DoubleRow restructures
(v7 best at 141 us, rel err 3.4e-3; v4 is broken).  Numerical traps found
on the way: tensor_scalar accum_out reduces with op1 and applies scalar2
only ONCE (not per element), and plain fp8 rounding of W2 fails the 2e-2
gate (2.3e-2) while error diffusion down the contraction axis passes.
"""

import os

import numpy as np

# Problem constants (hardcoded; kernel.py must be self-contained).
N = 400000
IN, H, F, S_STATIC, OUT = 64, 256, 128, 16, 5
N_CORES = 8
R = N // N_CORES  # rows per core = 50000
BLK = 500  # matmul moving free dim
NBLK = R // BLK  # 100
NPAIR = NBLK // 2  # 50

MODE = os.environ.get("DQN_MODE", "v3")

_prog_cache: dict = {}


def _build(mode: str, iters: int = 1):
    if mode == "v4":
        return _build_v4(iters)
    if mode == "v5":
        return _build_v5(iters)
    if mode == "v6":
        return _build_v6(iters)
    if mode == "v7":
        return _build_v7(iters)
    if mode == "v8":
        return _build_v8(iters)
    if mode == "v10":
        return _build_v10(iters)
    if mode == "v11":
        return _build_v11(iters)
    if mode == "v12":
        return _build_v13(iters, mixed=True)
    if mode == "v13":
        return _build_v13(iters, mixed=False)
    if mode in ("f16", "f32r", "f32r_split", "f32"):
        return _build_base(mode, iters)
    import concourse.mybir as mybir
    import concourse.tile as tile
    from concourse import bacc
    from contextlib import ExitStack

    dt = mybir.dt
    f32 = dt.float32
    f16 = dt.float16
    fp8 = mode == "v3fp8"
    h1_dt = dt.float8e4 if fp8 else f16

    nc = bacc.Bacc(
        "TRN2",
        target_bir_lowering=False,
        debug=False,
        enable_asserts=False,
        num_devices=1,
    )

    d_xt = nc.dram_tensor("d_xt", [IN, R], f16, kind="ExternalInput").ap()
    d_w1 = nc.dram_tensor("d_w1", [IN, H], f16, kind="ExternalInput").ap()
    if fp8:
        # packed [k, pair, m]: W2p[k, i, m] = W2q[128*i + k, m]
        d_w2 = nc.dram_tensor("d_w2", [128, 2, H], dt.float8e4, kind="ExternalInput").ap()
    else:
        d_w2 = nc.dram_tensor("d_w2", [H, H], f16, kind="ExternalInput").ap()
    # f32 per-partition biases: cols = [b1_h0, b1_h1, b2_h0, b2_h1]
    d_b = nc.dram_tensor("d_b", [128, 4], f32, kind="ExternalInput").ap()
    d_s = nc.dram_tensor("d_s", [128, 2], f32, kind="ExternalOutput").ap()

    Relu = mybir.ActivationFunctionType.Relu
    Alu = mybir.AluOpType
    X = mybir.AxisListType.X

    with tile.TileContext(nc) as tc, ExitStack() as ctx:
        cpool = ctx.enter_context(tc.tile_pool(name="cpool", bufs=1))
        xpool = ctx.enter_context(tc.tile_pool(name="xpool", bufs=3))
        hpool = ctx.enter_context(tc.tile_pool(name="hpool", bufs=2))
        spool = ctx.enter_context(tc.tile_pool(name="spool", bufs=1))
        ps1p = ctx.enter_context(tc.tile_pool(name="ps1p", bufs=1, space="PSUM"))
        ps2p = ctx.enter_context(tc.tile_pool(name="ps2p", bufs=1, space="PSUM"))

        # Constants resident in SBUF.
        w1_sb = cpool.tile([IN, H], f16, name="w1_sb")
        nc.sync.dma_start(w1_sb[:], d_w1)
        if fp8:
            w2p_sb = cpool.tile([128, 2, H], dt.float8e4, name="w2p_sb")
            nc.sync.dma_start(w2p_sb[:], d_w2)
        else:
            w2_sb = []
            for k in range(2):
                t = cpool.tile([128, H], f16, name=f"w2_sb{k}")
                nc.sync.dma_start(t[:], d_w2[k * 128 : (k + 1) * 128, :])
                w2_sb.append(t)
        bv = cpool.tile([128, 4], f32, name="bv")
        nc.sync.dma_start(bv[:], d_b)

        # Per-pair accumulated row-sums of relu(h2), one column per pair.
        acc = [cpool.tile([128, NPAIR], f32, name=f"acc{m}") for m in range(2)]

        for pair in [p for _ in range(iters) for p in range(NPAIR)]:
            xt = xpool.tile([IN, 2 * BLK], f16, name="xt", tag="xt")
            nc.sync.dma_start(xt[:], d_xt[:, pair * 2 * BLK : (pair + 1) * 2 * BLK])

            ps1 = [
                ps1p.tile([128, 2, 512], f32, name=f"ps1_{m}", tag=f"ps1_{m}")
                for m in range(2)
            ]
            ps2 = [
                ps2p.tile([128, 2, 512], f32, name=f"ps2_{m}", tag=f"ps2_{m}")
                for m in range(2)
            ]

            # Layer 1: 4 K=64 matmuls into half-major pair psum.
            for j in range(2):
                xr = xt[:, j * BLK : (j + 1) * BLK]
                for m in range(2):
                    nc.tensor.matmul(
                        ps1[m][:, j, 0:BLK],
                        w1_sb[:, m * 128 : (m + 1) * 128],
                        xr,
                        start=True,
                        stop=True,
                    )

            # h1 = relu(ps1 + b1): one DVE op per half (uniform bias vector).
            h1 = hpool.tile([128, 2, 2, 512], h1_dt, name="h1", tag="h1")
            for m in range(2):
                nc.vector.tensor_scalar(
                    h1[:, m, :, 0:BLK],
                    ps1[m][:, :, 0:BLK],
                    bv[:, m : m + 1],
                    0.0,
                    op0=Alu.add,
                    op1=Alu.max,
                )

            # Layer 2 into pair-level psum.
            for j in range(2):
                if fp8:
                    for m in range(2):
                        nc.tensor.matmul(
                            ps2[m][:, j, 0:BLK],
                            w2p_sb[:, :, m * 128 : (m + 1) * 128],
                            h1[:, :, j, 0:BLK],
                            start=True,
                            stop=True,
                            perf_mode=mybir.MatmulPerfMode.DoubleRow,
                        )
                else:
                    for m in range(2):
                        for k in range(2):
                            nc.tensor.matmul(
                                ps2[m][:, j, 0:BLK],
                                w2_sb[k][:, m * 128 : (m + 1) * 128],
                                h1[:, k, j, 0:BLK],
                                start=(k == 0),
                                stop=(k == 1),
                            )

            # relu(ps2 + b2) with fused row-sum; channels preserved because
            # each op spans one half of both blocks.
            for m in range(2):
                scr = spool.tile([128, 2, 512], f16, name=f"scr{m}", tag=f"scr{m}")
                nc.scalar.activation(
                    scr[:, :, 0:BLK],
                    ps2[m][:, :, 0:BLK],
                    Relu,
                    bias=bv[:, 2 + m : 3 + m],
                    accum_out=acc[m][:, pair : pair + 1],
                )

        s_sb = cpool.tile([128, 2], f32, name="s_sb")
        for m in range(2):
            nc.vector.reduce_sum(s_sb[:, m : m + 1], acc[m][:], axis=X)
        nc.sync.dma_start(d_s, s_sb[:])

    nc.compile()
    return nc


def _build_v4(iters: int = 1):
    """ACT-centric fp8 variant.

    Empirical per-op costs (probe.py, chained, psum-f32 src):
      ACT  = ~383 + 0.25*FD ns   (f16 out; 4x-packed stream)
      DVE  = ~397 + 0.71*FD ns
    so ACT is the cheap drain and op count is what matters.  Per 1000-row
    pair: ONE ACT op does relu(ps1) for all four [half,block] layer-1 banks
    (FD=2000; b1 pre-added by K=1 ones-matmuls on PE strips 2-3, concurrent
    with the K=64 layer-1 matmuls on strips 0-1); layer-2 relu+accum runs
    half0 on ACT, half1 on DVE (bias as per-partition vector operands).
    Layer 2 is 2 fp8 DoubleRow matmuls per block (K_eff=256), weights-outer
    so LDWEIGHTS amortizes over the pair.
    """
    import concourse.mybir as mybir
    import concourse.tile as tile
    from concourse import bacc
    from contextlib import ExitStack

    dt = mybir.dt
    f32 = dt.float32
    f16 = dt.float16
    fp8 = dt.float8e4
    Relu = mybir.ActivationFunctionType.Relu
    Alu = mybir.AluOpType
    X = mybir.AxisListType.X

    nc = bacc.Bacc(
        "TRN2",
        target_bir_lowering=False,
        debug=False,
        enable_asserts=False,
        num_devices=1,
    )

    d_xt = nc.dram_tensor("d_xt", [IN, R], f16, kind="ExternalInput").ap()
    d_w1 = nc.dram_tensor("d_w1", [IN, H], f16, kind="ExternalInput").ap()
    d_w2 = nc.dram_tensor("d_w2", [128, 2, H], fp8, kind="ExternalInput").ap()
    # f16 b1 halves for the ones-matmuls, rows 64/96; f32 b2 via vector ops.
    d_b1 = nc.dram_tensor("d_b1", [128, 128], f16, kind="ExternalInput").ap()
    d_b2 = nc.dram_tensor("d_b2", [128, 2], f32, kind="ExternalInput").ap()
    d_s = nc.dram_tensor("d_s", [128, 2], f32, kind="ExternalOutput").ap()

    with tile.TileContext(nc) as tc, ExitStack() as ctx:
        cpool = ctx.enter_context(tc.tile_pool(name="cpool", bufs=1))
        xpool = ctx.enter_context(tc.tile_pool(name="xpool", bufs=3))
        hpool = ctx.enter_context(tc.tile_pool(name="hpool", bufs=2))
        spool = ctx.enter_context(tc.tile_pool(name="spool", bufs=1))
        ps1p = ctx.enter_context(tc.tile_pool(name="ps1p", bufs=1, space="PSUM"))
        ps2p = ctx.enter_context(tc.tile_pool(name="ps2p", bufs=1, space="PSUM"))

        w1_sb = cpool.tile([IN, H], f16, name="w1_sb")
        nc.sync.dma_start(w1_sb[:], d_w1)
        w2p_sb = cpool.tile([128, 2, H], fp8, name="w2p_sb")
        nc.sync.dma_start(w2p_sb[:], d_w2)
        b1_sb = cpool.tile([128, 128], f16, name="b1_sb")
        nc.sync.dma_start(b1_sb[:], d_b1)
        b2_sb = cpool.tile([128, 2], f32, name="b2_sb")
        nc.sync.dma_start(b2_sb[:], d_b2)
        nb2 = cpool.tile([128, 1], f32, name="nb2")
        nc.vector.tensor_scalar_mul(nb2[:], b2_sb[:, 1:2], -1.0)
        ones_sb = cpool.tile([128, BLK], f16, name="ones_sb")
        nc.vector.memset(ones_sb[:], 1.0)

        acc = [cpool.tile([128, NPAIR], f32, name=f"acc{m}") for m in range(2)]

        for pair in [p for _ in range(iters) for p in range(NPAIR)]:
            xt = xpool.tile([IN, 2 * BLK], f16, name="xt", tag="xt")
            nc.sync.dma_start(xt[:], d_xt[:, pair * 2 * BLK : (pair + 1) * 2 * BLK])

            # ps1: [half, block] banks, 4 banks, one tile per pair.
            ps1 = ps1p.tile([128, 2, 2, 512], f32, name="ps1", tag="ps1")
            for j in range(2):
                xr = xt[:, j * BLK : (j + 1) * BLK]
                for m in range(2):
                    strip = 64 if m == 0 else 96
                    nc.tensor.matmul(
                        ps1[:, m, j, 0:BLK],
                        b1_sb[strip : strip + 1, 0:128],
                        ones_sb[strip : strip + 1, 0:BLK],
                        start=True,
                        stop=False,
                        tile_position=(strip, 0),
                        skip_group_check=True,
                    )
                    nc.tensor.matmul(
                        ps1[:, m, j, 0:BLK],
                        w1_sb[:, m * 128 : (m + 1) * 128],
                        xr,
                        start=False,
                        stop=True,
                        skip_group_check=True,
                    )

            # (a): one ACT op drains all of ps1 -> packed fp8 h1.
            h1 = hpool.tile([128, 2, 2, 512], fp8, name="h1", tag="h1")
            nc.scalar.activation(h1[:, :, :, 0:BLK], ps1[:, :, :, 0:BLK], Relu)

            # Layer 2: DoubleRow, weights-outer so each half's LDWEIGHTS is
            # shared by both blocks of the pair.
            ps2 = [
                ps2p.tile([128, 2, 512], f32, name=f"ps2_{m}", tag=f"ps2_{m}")
                for m in range(2)
            ]
            for m in range(2):
                for j in range(2):
                    nc.tensor.matmul(
                        ps2[m][:, j, 0:BLK],
                        w2p_sb[:, :, m * 128 : (m + 1) * 128],
                        h1[:, :, j, 0:BLK],
                        start=True,
                        stop=True,
                        perf_mode=mybir.MatmulPerfMode.DoubleRow,
                    )

            # (b): relu(ps2 + b2) + per-channel row-sum; half0 on ACT,
            # half1 on DVE so the two drains run in parallel.
            scr0 = spool.tile([128, 2, 512], f16, name="scr0", tag="scr0")
            nc.scalar.activation(
                scr0[:, :, 0:BLK],
                ps2[0][:, :, 0:BLK],
                Relu,
                bias=b2_sb[:, 0:1],
                accum_out=acc[0][:, pair : pair + 1],
            )
            scr1 = spool.tile([128, 2, 512], f16, name="scr1", tag="scr1")
            nc.vector.tensor_scalar(
                scr1[:, :, 0:BLK],
                ps2[1][:, :, 0:BLK],
                nb2[:],
                None,
                op0=Alu.max,
                op1=Alu.add,
                accum_out=acc[1][:, pair : pair + 1],
            )

        s_sb = cpool.tile([128, 2], f32, name="s_sb")
        for m in range(2):
            nc.vector.reduce_sum(s_sb[:, m : m + 1], acc[m][:], axis=X)
        nc.sync.dma_start(d_s, s_sb[:])

    nc.compile()
    return nc


def _build_v5(iters: int = 1):
    """fp8 DoubleRow layer 2 with probe-informed engine split.

    Empirical per-op costs (probe.py, chained, psum-f32 src, FD=1000):
      ACT relu+bias(+accum) ~633 ns ;  DVE 2-op(+accum) ~1267 ns
    Per 1000-row pair (ops all pair-level, half-major so the per-partition
    bias vector is uniform within each op):
      ACT: (a)h0, (a)h1  relu(ps1+b1)->fp8 h1,  (b)h0 relu+accum  ~1.9 us
      DVE: (b)h1 relu+accum                                       ~1.3 us
      PE : 4x K=64 f16 layer-1 mm + 4x DoubleRow K_eff=256 layer-2 mm
           (weights-outer so each half's LDWEIGHTS covers both blocks)
    """
    import concourse.mybir as mybir
    import concourse.tile as tile
    from concourse import bacc
    from contextlib import ExitStack

    dt = mybir.dt
    f32 = dt.float32
    f16 = dt.float16
    fp8 = dt.float8e4
    Relu = mybir.ActivationFunctionType.Relu
    Alu = mybir.AluOpType
    X = mybir.AxisListType.X

    nc = bacc.Bacc(
        "TRN2",
        target_bir_lowering=False,
        debug=False,
        enable_asserts=False,
        num_devices=1,
    )

    d_xt = nc.dram_tensor("d_xt", [IN, R], f16, kind="ExternalInput").ap()
    d_w1 = nc.dram_tensor("d_w1", [IN, H], f16, kind="ExternalInput").ap()
    d_w2 = nc.dram_tensor("d_w2", [128, 2, H], fp8, kind="ExternalInput").ap()
    # f32 per-partition biases: cols = [b1_h0, b1_h1, b2_h0, b2_h1]
    d_b = nc.dram_tensor("d_b", [128, 4], f32, kind="ExternalInput").ap()
    d_s = nc.dram_tensor("d_s", [128, 2], f32, kind="ExternalOutput").ap()

    with tile.TileContext(nc) as tc, ExitStack() as ctx:
        cpool = ctx.enter_context(tc.tile_pool(name="cpool", bufs=1))
        xpool = ctx.enter_context(tc.tile_pool(name="xpool", bufs=3))
        hpool = ctx.enter_context(tc.tile_pool(name="hpool", bufs=2))
        spool = ctx.enter_context(tc.tile_pool(name="spool", bufs=1))
        ps1p = ctx.enter_context(tc.tile_pool(name="ps1p", bufs=1, space="PSUM"))
        ps2p = ctx.enter_context(tc.tile_pool(name="ps2p", bufs=1, space="PSUM"))

        w1_sb = cpool.tile([IN, H], f16, name="w1_sb")
        nc.sync.dma_start(w1_sb[:], d_w1)
        w2p_sb = cpool.tile([128, 2, H], fp8, name="w2p_sb")
        nc.sync.dma_start(w2p_sb[:], d_w2)
        bv = cpool.tile([128, 4], f32, name="bv")
        nc.sync.dma_start(bv[:], d_b)
        # negated b2_h1 for the DVE path: out = max(ps2, -b2) + b2, so the
        # accum reduce op (== op1) is add and accum_out is a true sum.
        nb2 = cpool.tile([128, 1], f32, name="nb2")
        nc.vector.tensor_scalar_mul(nb2[:], bv[:, 3:4], -1.0)

        acc = [cpool.tile([128, NPAIR], f32, name=f"acc{m}") for m in range(2)]

        for pair in [p for _ in range(iters) for p in range(NPAIR)]:
            xt = xpool.tile([IN, 2 * BLK], f16, name="xt", tag="xt")
            nc.sync.dma_start(xt[:], d_xt[:, pair * 2 * BLK : (pair + 1) * 2 * BLK])

            ps1 = [
                ps1p.tile([128, 2, 512], f32, name=f"ps1_{m}", tag=f"ps1_{m}")
                for m in range(2)
            ]
            for j in range(2):
                xr = xt[:, j * BLK : (j + 1) * BLK]
                for m in range(2):
                    nc.tensor.matmul(
                        ps1[m][:, j, 0:BLK],
                        w1_sb[:, m * 128 : (m + 1) * 128],
                        xr,
                        start=True,
                        stop=True,
                    )

            # (a): h1 = relu(ps1 + b1) -> packed fp8, one ACT op per half.
            h1 = hpool.tile([128, 2, 2, 512], fp8, name="h1", tag="h1")
            for m in range(2):
                nc.scalar.activation(
                    h1[:, m, :, 0:BLK],
                    ps1[m][:, :, 0:BLK],
                    Relu,
                    bias=bv[:, m : m + 1],
                )

            # Layer 2: DoubleRow, weights-outer so each half's LDWEIGHTS is
            # shared by both blocks of the pair.
            ps2 = [
                ps2p.tile([128, 2, 512], f32, name=f"ps2_{m}", tag=f"ps2_{m}")
                for m in range(2)
            ]
            for m in range(2):
                for j in range(2):
                    nc.tensor.matmul(
                        ps2[m][:, j, 0:BLK],
                        w2p_sb[:, :, m * 128 : (m + 1) * 128],
                        h1[:, :, j, 0:BLK],
                        start=True,
                        stop=True,
                        perf_mode=mybir.MatmulPerfMode.DoubleRow,
                    )

            # (b): relu(ps2 + b2) + per-channel row-sum; half0 on ACT,
            # half1 on DVE so the two drains run in parallel.
            scr0 = spool.tile([128, 2, 512], f16, name="scr0", tag="scr0")
            nc.scalar.activation(
                scr0[:, :, 0:BLK],
                ps2[0][:, :, 0:BLK],
                Relu,
                bias=bv[:, 2:3],
                accum_out=acc[0][:, pair : pair + 1],
            )
            scr1 = spool.tile([128, 2, 512], f16, name="scr1", tag="scr1")
            nc.vector.tensor_scalar(
                scr1[:, :, 0:BLK],
                ps2[1][:, :, 0:BLK],
                nb2[:],
                None,
                op0=Alu.max,
                op1=Alu.add,
                accum_out=acc[1][:, pair : pair + 1],
            )

        s_sb = cpool.tile([128, 2], f32, name="s_sb")
        for m in range(2):
            nc.vector.reduce_sum(s_sb[:, m : m + 1], acc[m][:], axis=X)
        nc.sync.dma_start(d_s, s_sb[:])

    nc.compile()
    return nc


def _build_v6(iters: int = 1):
    """Like v5 but layer-1 bias rides in the matmul contraction (K=65
    ones-row, as in the original baseline), so layer-1 relu needs no bias
    and collapses to ONE ACT op per pair over a single 4-bank psum tile
    with slot index (2*block + half):
      ACT: (a) relu(ps1)->fp8 h1 FD=2000,  (b)h0 relu+bias+accum FD=1000
      DVE: (b)h1 add-bias+max+accum FD=1000
      PE : 4x K=65 f16 layer-1 mm + 4x DoubleRow layer-2 mm per pair
    """
    import concourse.mybir as mybir
    import concourse.tile as tile
    from concourse import bacc
    from contextlib import ExitStack

    dt = mybir.dt
    f32 = dt.float32
    f16 = dt.float16
    fp8 = dt.float8e4
    Relu = mybir.ActivationFunctionType.Relu
    Alu = mybir.AluOpType
    X = mybir.AxisListType.X

    nc = bacc.Bacc(
        "TRN2",
        target_bir_lowering=False,
        debug=False,
        enable_asserts=False,
        num_devices=1,
    )

    d_xt = nc.dram_tensor("d_xt", [IN + 1, R], f16, kind="ExternalInput").ap()
    d_w1 = nc.dram_tensor("d_w1", [IN + 1, H], f16, kind="ExternalInput").ap()
    d_w2 = nc.dram_tensor("d_w2", [128, 2, H], fp8, kind="ExternalInput").ap()
    d_b2 = nc.dram_tensor("d_b2", [128, 2], f32, kind="ExternalInput").ap()
    d_s = nc.dram_tensor("d_s", [128, 2], f32, kind="ExternalOutput").ap()

    with tile.TileContext(nc) as tc, ExitStack() as ctx:
        cpool = ctx.enter_context(tc.tile_pool(name="cpool", bufs=1))
        xpool = ctx.enter_context(tc.tile_pool(name="xpool", bufs=3))
        hpool = ctx.enter_context(tc.tile_pool(name="hpool", bufs=2))
        spool = ctx.enter_context(tc.tile_pool(name="spool", bufs=1))
        ps1p = ctx.enter_context(tc.tile_pool(name="ps1p", bufs=1, space="PSUM"))
        ps2p = ctx.enter_context(tc.tile_pool(name="ps2p", bufs=1, space="PSUM"))

        w1_sb = cpool.tile([IN + 1, H], f16, name="w1_sb")
        nc.sync.dma_start(w1_sb[:], d_w1)
        w2p_sb = cpool.tile([128, 2, H], fp8, name="w2p_sb")
        nc.sync.dma_start(w2p_sb[:], d_w2)
        b2_sb = cpool.tile([128, 2], f32, name="b2_sb")
        nc.sync.dma_start(b2_sb[:], d_b2)
        # negated b2_h1 for the DVE path: out = max(ps2, -b2) + b2, so the
        # accum reduce op (== op1) is add and accum_out is a true sum.
        nb2 = cpool.tile([128, 1], f32, name="nb2")
        nc.vector.tensor_scalar_mul(nb2[:], b2_sb[:, 1:2], -1.0)

        acc = [cpool.tile([128, NPAIR], f32, name=f"acc{m}") for m in range(2)]

        for pair in [p for _ in range(iters) for p in range(NPAIR)]:
            xt = xpool.tile([IN + 1, 2 * BLK], f16, name="xt", tag="xt")
            nc.sync.dma_start(xt[:], d_xt[:, pair * 2 * BLK : (pair + 1) * 2 * BLK])

            # ps1 slots: index 2*block + half.
            ps1 = ps1p.tile([128, 4, 512], f32, name="ps1", tag="ps1")
            for j in range(2):
                xr = xt[:, j * BLK : (j + 1) * BLK]
                for m in range(2):
                    nc.tensor.matmul(
                        ps1[:, 2 * j + m, 0:BLK],
                        w1_sb[:, m * 128 : (m + 1) * 128],
                        xr,
                        start=True,
                        stop=True,
                    )

            # (a): ONE ACT op drains all four layer-1 banks -> packed fp8.
            h1 = hpool.tile([128, 4, 512], fp8, name="h1", tag="h1")
            nc.scalar.activation(h1[:, :, 0:BLK], ps1[:, :, 0:BLK], Relu)

            # Layer 2: DoubleRow, weights-outer; rhs pair-dim = half
            # (slots 2j..2j+1 are the two halves of block j).
            ps2 = [
                ps2p.tile([128, 2, 512], f32, name=f"ps2_{m}", tag=f"ps2_{m}")
                for m in range(2)
            ]
            for m in range(2):
                for j in range(2):
                    nc.tensor.matmul(
                        ps2[m][:, j, 0:BLK],
                        w2p_sb[:, :, m * 128 : (m + 1) * 128],
                        h1[:, 2 * j : 2 * j + 2, 0:BLK],
                        start=True,
                        stop=True,
                        perf_mode=mybir.MatmulPerfMode.DoubleRow,
                    )

            # (b): relu(ps2 + b2) + per-channel row-sum; half0 on ACT,
            # half1 on DVE.
            scr0 = spool.tile([128, 2, 512], f16, name="scr0", tag="scr0")
            nc.scalar.activation(
                scr0[:, :, 0:BLK],
                ps2[0][:, :, 0:BLK],
                Relu,
                bias=b2_sb[:, 0:1],
                accum_out=acc[0][:, pair : pair + 1],
            )
            scr1 = spool.tile([128, 2, 512], f16, name="scr1", tag="scr1")
            nc.vector.tensor_scalar(
                scr1[:, :, 0:BLK],
                ps2[1][:, :, 0:BLK],
                nb2[:],
                None,
                op0=Alu.max,
                op1=Alu.add,
                accum_out=acc[1][:, pair : pair + 1],
            )

        s_sb = cpool.tile([128, 2], f32, name="s_sb")
        for m in range(2):
            nc.vector.reduce_sum(s_sb[:, m : m + 1], acc[m][:], axis=X)
        nc.sync.dma_start(d_s, s_sb[:])

    nc.compile()
    return nc


def _build_v8(iters: int = 1):
    """v5 with strip-concurrent layer 1.

    Layer-1 matmuls are K=64 and use only PE row strips 0-1, so two of them
    run CONCURRENTLY when placed at tile_position (0,0) and (64,0): the host
    interleaves the pair's two 500-row blocks across partition halves
    (block A features in partitions 0-63, block B in 64-127) and W1 is
    duplicated across both halves.  Each pass computes one h-half of block A
    alongside the other h-half of block B, so layer 1 costs ~1000 PE cycles
    per pair instead of 2000.  Everything else is v5: fp8 DoubleRow layer 2,
    (a) = 2 pair-level ACT relu+bias ops, (b) = ACT half0 / DVE half1 (DVE
    accumulates sum(max(ps2,-b2)); + b2*N lands in the host tail).
    """
    import concourse.mybir as mybir
    import concourse.tile as tile
    from concourse import bacc
    from contextlib import ExitStack

    dt = mybir.dt
    f32 = dt.float32
    f16 = dt.float16
    fp8 = dt.float8e4
    Relu = mybir.ActivationFunctionType.Relu
    Alu = mybir.AluOpType
    X = mybir.AxisListType.X

    R2 = R // 2  # columns of the interleaved x layout

    nc = bacc.Bacc(
        "TRN2",
        target_bir_lowering=False,
        debug=False,
        enable_asserts=False,
        num_devices=1,
    )

    d_xt = nc.dram_tensor("d_xt", [128, R2], f16, kind="ExternalInput").ap()
    d_w1 = nc.dram_tensor("d_w1", [128, H], f16, kind="ExternalInput").ap()
    d_w2 = nc.dram_tensor("d_w2", [128, 2, H], fp8, kind="ExternalInput").ap()
    d_b = nc.dram_tensor("d_b", [128, 4], f32, kind="ExternalInput").ap()
    d_s = nc.dram_tensor("d_s", [128, 2], f32, kind="ExternalOutput").ap()

    with tile.TileContext(nc) as tc, ExitStack() as ctx:
        cpool = ctx.enter_context(tc.tile_pool(name="cpool", bufs=1))
        xpool = ctx.enter_context(tc.tile_pool(name="xpool", bufs=3))
        hpool = ctx.enter_context(tc.tile_pool(name="hpool", bufs=3))
        spool = ctx.enter_context(tc.tile_pool(name="spool", bufs=1))
        ps1p = ctx.enter_context(tc.tile_pool(name="ps1p", bufs=1, space="PSUM"))
        ps2p = ctx.enter_context(tc.tile_pool(name="ps2p", bufs=1, space="PSUM"))

        # W1 duplicated across both partition halves: [128, 256] f16.
        w1_sb = cpool.tile([128, H], f16, name="w1_sb")
        nc.sync.dma_start(w1_sb[:], d_w1)
        w2p_sb = cpool.tile([128, 2, H], fp8, name="w2p_sb")
        nc.sync.dma_start(w2p_sb[:], d_w2)
        bv = cpool.tile([128, 4], f32, name="bv")
        nc.sync.dma_start(bv[:], d_b)
        nb2 = cpool.tile([128, 1], f32, name="nb2")
        nc.vector.tensor_scalar_mul(nb2[:], bv[:, 3:4], -1.0)

        acc = [cpool.tile([128, NPAIR], f32, name=f"acc{m}") for m in range(2)]

        for pair in [p for _ in range(iters) for p in range(NPAIR)]:
            # [128, 500]: partitions 0-63 = block A features, 64-127 = block B.
            xt = xpool.tile([128, BLK], f16, name="xt", tag="xt")
            nc.sync.dma_start(xt[:], d_xt[:, pair * BLK : (pair + 1) * BLK])

            ps1 = [
                ps1p.tile([128, 2, 512], f32, name=f"ps1_{m}", tag=f"ps1_{m}")
                for m in range(2)
            ]
            # Pass 1: A-h0 on strips 0-1 || B-h1 on strips 2-3.
            nc.tensor.matmul(
                ps1[0][:, 0, 0:BLK], w1_sb[0:64, 0:128], xt[0:64, 0:BLK],
                start=True, stop=True,
            )
            nc.tensor.matmul(
                ps1[1][:, 1, 0:BLK], w1_sb[64:128, 128:256], xt[64:128, 0:BLK],
                start=True, stop=True,
            )
            # Pass 2: A-h1 || B-h0.
            nc.tensor.matmul(
                ps1[1][:, 0, 0:BLK], w1_sb[0:64, 128:256], xt[0:64, 0:BLK],
                start=True, stop=True,
            )
            nc.tensor.matmul(
                ps1[0][:, 1, 0:BLK], w1_sb[64:128, 0:128], xt[64:128, 0:BLK],
                start=True, stop=True,
            )

            # (a): h1 = relu(ps1 + b1) -> packed fp8, one ACT op per half.
            h1 = hpool.tile([128, 2, 2, 512], fp8, name="h1", tag="h1")
            for m in range(2):
                nc.scalar.activation(
                    h1[:, m, :, 0:BLK],
                    ps1[m][:, :, 0:BLK],
                    Relu,
                    bias=bv[:, m : m + 1],
                )

            ps2 = [
                ps2p.tile([128, 2, 512], f32, name=f"ps2_{m}", tag=f"ps2_{m}")
                for m in range(2)
            ]
            for j in range(2):
                for m in range(2):
                    nc.tensor.matmul(
                        ps2[m][:, j, 0:BLK],
                        w2p_sb[:, :, m * 128 : (m + 1) * 128],
                        h1[:, :, j, 0:BLK],
                        start=True,
                        stop=True,
                        perf_mode=mybir.MatmulPerfMode.DoubleRow,
                    )
            # keep-warm blips: tiny matmuls into psum padding keep PE
            # activity in every HAM window (~50 ns each, no readers).
            nc.tensor.matmul(
                ps1[0][0:8, 0, 500:512], w1_sb[0:1, 0:8], w1_sb[0:1, 0:12],
                start=True, stop=True,
            )
            nc.tensor.matmul(
                ps1[1][0:8, 0, 500:512], w1_sb[0:1, 0:8], w1_sb[0:1, 0:12],
                start=True, stop=True,
            )

            scr0 = spool.tile([128, 2, 512], f16, name="scr0", tag="scr0")
            nc.scalar.activation(
                scr0[:, :, 0:BLK],
                ps2[0][:, :, 0:BLK],
                Relu,
                bias=bv[:, 2:3],
                accum_out=acc[0][:, pair : pair + 1],
            )
            scr1 = spool.tile([128, 2, 512], f16, name="scr1", tag="scr1")
            nc.vector.tensor_scalar(
                scr1[:, :, 0:BLK],
                ps2[1][:, :, 0:BLK],
                nb2[:],
                None,
                op0=Alu.max,
                op1=Alu.add,
                accum_out=acc[1][:, pair : pair + 1],
            )

        s_sb = cpool.tile([128, 2], f32, name="s_sb")
        for m in range(2):
            nc.vector.reduce_sum(s_sb[:, m : m + 1], acc[m][:], axis=X)
        nc.sync.dma_start(d_s, s_sb[:])

    nc.compile()
    return nc


def _build_v7(iters: int = 1):
    """v5 with strip-concurrent layer 1.

    Layer-1 matmuls are K=64 and use only PE row strips 0-1, so two of them
    run CONCURRENTLY when placed at tile_position (0,0) and (64,0): the host
    interleaves the pair's two 500-row blocks across partition halves
    (block A features in partitions 0-63, block B in 64-127) and W1 is
    duplicated across both halves.  Each pass computes one h-half of block A
    alongside the other h-half of block B, so layer 1 costs ~1000 PE cycles
    per pair instead of 2000.  Everything else is v5: fp8 DoubleRow layer 2,
    (a) = 2 pair-level ACT relu+bias ops, (b) = ACT half0 / DVE half1 (DVE
    accumulates sum(max(ps2,-b2)); + b2*N lands in the host tail).
    """
    import concourse.mybir as mybir
    import concourse.tile as tile
    from concourse import bacc
    from contextlib import ExitStack

    dt = mybir.dt
    f32 = dt.float32
    f16 = dt.float16
    fp8 = dt.float8e4
    Relu = mybir.ActivationFunctionType.Relu
    Alu = mybir.AluOpType
    X = mybir.AxisListType.X

    R2 = R // 2  # columns of the interleaved x layout

    nc = bacc.Bacc(
        "TRN2",
        target_bir_lowering=False,
        debug=False,
        enable_asserts=False,
        num_devices=1,
    )

    d_xt = nc.dram_tensor("d_xt", [128, R2], f16, kind="ExternalInput").ap()
    d_w1 = nc.dram_tensor("d_w1", [128, H], f16, kind="ExternalInput").ap()
    d_w2 = nc.dram_tensor("d_w2", [128, 2, H], fp8, kind="ExternalInput").ap()
    d_b = nc.dram_tensor("d_b", [128, 4], f32, kind="ExternalInput").ap()
    d_s = nc.dram_tensor("d_s", [128, 2], f32, kind="ExternalOutput").ap()

    with tile.TileContext(nc) as tc, ExitStack() as ctx:
        cpool = ctx.enter_context(tc.tile_pool(name="cpool", bufs=1))
        xpool = ctx.enter_context(tc.tile_pool(name="xpool", bufs=3))
        hpool = ctx.enter_context(tc.tile_pool(name="hpool", bufs=2))
        spool = ctx.enter_context(tc.tile_pool(name="spool", bufs=1))
        ps1p = ctx.enter_context(tc.tile_pool(name="ps1p", bufs=1, space="PSUM"))
        ps2p = ctx.enter_context(tc.tile_pool(name="ps2p", bufs=1, space="PSUM"))

        # W1 duplicated across both partition halves: [128, 256] f16.
        w1_sb = cpool.tile([128, H], f16, name="w1_sb")
        nc.sync.dma_start(w1_sb[:], d_w1)
        w2p_sb = cpool.tile([128, 2, H], fp8, name="w2p_sb")
        nc.sync.dma_start(w2p_sb[:], d_w2)
        bv = cpool.tile([128, 4], f32, name="bv")
        nc.sync.dma_start(bv[:], d_b)
        nb2 = cpool.tile([128, 1], f32, name="nb2")
        nc.vector.tensor_scalar_mul(nb2[:], bv[:, 3:4], -1.0)

        acc = [cpool.tile([128, NPAIR], f32, name=f"acc{m}") for m in range(2)]

        for pair in [p for _ in range(iters) for p in range(NPAIR)]:
            # [128, 500]: partitions 0-63 = block A features, 64-127 = block B.
            xt = xpool.tile([128, BLK], f16, name="xt", tag="xt")
            nc.sync.dma_start(xt[:], d_xt[:, pair * BLK : (pair + 1) * BLK])

            ps1 = [
                ps1p.tile([128, 2, 512], f32, name=f"ps1_{m}", tag=f"ps1_{m}")
                for m in range(2)
            ]
            # Pass 1: A-h0 on strips 0-1 || B-h1 on strips 2-3.
            nc.tensor.matmul(
                ps1[0][:, 0, 0:BLK], w1_sb[0:64, 0:128], xt[0:64, 0:BLK],
                start=True, stop=True,
            )
            nc.tensor.matmul(
                ps1[1][:, 1, 0:BLK], w1_sb[64:128, 128:256], xt[64:128, 0:BLK],
                start=True, stop=True,
            )
            # Pass 2: A-h1 || B-h0.
            nc.tensor.matmul(
                ps1[1][:, 0, 0:BLK], w1_sb[0:64, 128:256], xt[0:64, 0:BLK],
                start=True, stop=True,
            )
            nc.tensor.matmul(
                ps1[0][:, 1, 0:BLK], w1_sb[64:128, 0:128], xt[64:128, 0:BLK],
                start=True, stop=True,
            )

            # (a): h1 = relu(ps1 + b1) -> packed fp8, one ACT op per half.
            h1 = hpool.tile([128, 2, 2, 512], fp8, name="h1", tag="h1")
            for m in range(2):
                nc.scalar.activation(
                    h1[:, m, :, 0:BLK],
                    ps1[m][:, :, 0:BLK],
                    Relu,
                    bias=bv[:, m : m + 1],
                )

            ps2 = [
                ps2p.tile([128, 2, 512], f32, name=f"ps2_{m}", tag=f"ps2_{m}")
                for m in range(2)
            ]
            for m in range(2):
                for j in range(2):
                    nc.tensor.matmul(
                        ps2[m][:, j, 0:BLK],
                        w2p_sb[:, :, m * 128 : (m + 1) * 128],
                        h1[:, :, j, 0:BLK],
                        start=True,
                        stop=True,
                        perf_mode=mybir.MatmulPerfMode.DoubleRow,
                    )

            scr0 = spool.tile([128, 2, 512], f16, name="scr0", tag="scr0")
            nc.scalar.activation(
                scr0[:, :, 0:BLK],
                ps2[0][:, :, 0:BLK],
                Relu,
                bias=bv[:, 2:3],
                accum_out=acc[0][:, pair : pair + 1],
            )
            scr1 = spool.tile([128, 2, 512], f16, name="scr1", tag="scr1")
            nc.vector.tensor_scalar(
                scr1[:, :, 0:BLK],
                ps2[1][:, :, 0:BLK],
                nb2[:],
                None,
                op0=Alu.max,
                op1=Alu.add,
                accum_out=acc[1][:, pair : pair + 1],
            )

        s_sb = cpool.tile([128, 2], f32, name="s_sb")
        for m in range(2):
            nc.vector.reduce_sum(s_sb[:, m : m + 1], acc[m][:], axis=X)
        nc.sync.dma_start(d_s, s_sb[:])

    nc.compile()
    return nc


def _build_v10(iters: int = 1):
    """v3 (warm all-f16) + strip-concurrent layer 1: same grouped f16
    layer-2 stream that keeps the PE un-throttled at 2.4 GHz, but layer-1's
    four K=64 matmuls run as two concurrent pairs at tile_position (0,0) /
    (64,0) with the pair's blocks interleaved across partition halves."""
    import concourse.mybir as mybir
    import concourse.tile as tile
    from concourse import bacc
    from contextlib import ExitStack

    dt = mybir.dt
    f32 = dt.float32
    f16 = dt.float16
    Relu = mybir.ActivationFunctionType.Relu
    Alu = mybir.AluOpType
    X = mybir.AxisListType.X
    R2 = R // 2

    nc = bacc.Bacc("TRN2", target_bir_lowering=False, debug=False,
                   enable_asserts=False, num_devices=1)

    d_xt = nc.dram_tensor("d_xt", [128, R2], f16, kind="ExternalInput").ap()
    d_w1 = nc.dram_tensor("d_w1", [128, H], f16, kind="ExternalInput").ap()
    d_w2 = nc.dram_tensor("d_w2", [H, H], f16, kind="ExternalInput").ap()
    d_b = nc.dram_tensor("d_b", [128, 4], f32, kind="ExternalInput").ap()
    d_s = nc.dram_tensor("d_s", [128, 2], f32, kind="ExternalOutput").ap()

    with tile.TileContext(nc) as tc, ExitStack() as ctx:
        cpool = ctx.enter_context(tc.tile_pool(name="cpool", bufs=1))
        xpool = ctx.enter_context(tc.tile_pool(name="xpool", bufs=3))
        hpool = ctx.enter_context(tc.tile_pool(name="hpool", bufs=2))
        spool = ctx.enter_context(tc.tile_pool(name="spool", bufs=1))
        ps1p = ctx.enter_context(tc.tile_pool(name="ps1p", bufs=1, space="PSUM"))
        ps2p = ctx.enter_context(tc.tile_pool(name="ps2p", bufs=1, space="PSUM"))

        w1_sb = cpool.tile([128, H], f16, name="w1_sb")
        nc.sync.dma_start(w1_sb[:], d_w1)
        w2_sb = []
        for k in range(2):
            t = cpool.tile([128, H], f16, name=f"w2_sb{k}")
            nc.sync.dma_start(t[:], d_w2[k * 128 : (k + 1) * 128, :])
            w2_sb.append(t)
        bv = cpool.tile([128, 4], f32, name="bv")
        nc.sync.dma_start(bv[:], d_b)

        acc = [cpool.tile([128, NPAIR], f32, name=f"acc{m}") for m in range(2)]

        for pair in [p for _ in range(iters) for p in range(NPAIR)]:
            xt = xpool.tile([128, BLK], f16, name="xt", tag="xt")
            nc.sync.dma_start(xt[:], d_xt[:, pair * BLK : (pair + 1) * BLK])

            ps1 = [
                ps1p.tile([128, 2, 512], f32, name=f"ps1_{m}", tag=f"ps1_{m}")
                for m in range(2)
            ]
            ps2 = [
                ps2p.tile([128, 2, 512], f32, name=f"ps2_{m}", tag=f"ps2_{m}")
                for m in range(2)
            ]

            # Layer 1: two concurrent passes; pass 1 completes ps1[0].
            nc.tensor.matmul(ps1[0][:, 0, 0:BLK], w1_sb[0:64, 0:128],
                             xt[0:64, 0:BLK], start=True, stop=True)
            nc.tensor.matmul(ps1[0][:, 1, 0:BLK], w1_sb[64:128, 0:128],
                             xt[64:128, 0:BLK], start=True, stop=True)
            nc.tensor.matmul(ps1[1][:, 0, 0:BLK], w1_sb[0:64, 128:256],
                             xt[0:64, 0:BLK], start=True, stop=True)
            nc.tensor.matmul(ps1[1][:, 1, 0:BLK], w1_sb[64:128, 128:256],
                             xt[64:128, 0:BLK], start=True, stop=True)

            h1 = hpool.tile([128, 2, 2, 512], f16, name="h1", tag="h1")
            for m in range(2):
                nc.vector.tensor_scalar(
                    h1[:, m, :, 0:BLK], ps1[m][:, :, 0:BLK],
                    bv[:, m : m + 1], 0.0, op0=Alu.add, op1=Alu.max,
                )

            for j in range(2):
                for m in range(2):
                    for k in range(2):
                        nc.tensor.matmul(
                            ps2[m][:, j, 0:BLK],
                            w2_sb[k][:, m * 128 : (m + 1) * 128],
                            h1[:, k, j, 0:BLK],
                            start=(k == 0),
                            stop=(k == 1),
                        )

            for m in range(2):
                scr = spool.tile([128, 2, 512], f16, name=f"scr{m}", tag=f"scr{m}")
                nc.scalar.activation(
                    scr[:, :, 0:BLK], ps2[m][:, :, 0:BLK], Relu,
                    bias=bv[:, 2 + m : 3 + m],
                    accum_out=acc[m][:, pair : pair + 1],
                )

        s_sb = cpool.tile([128, 2], f32, name="s_sb")
        for m in range(2):
            nc.vector.reduce_sum(s_sb[:, m : m + 1], acc[m][:], axis=X)
        nc.sync.dma_start(d_s, s_sb[:])

    nc.compile()
    return nc


def _build_base(mode: str, iters: int = 1, xbufs: int = 4, hbufs: int = 3):
    """The original staged baseline (f16 default): ones-row K=65 layer 1,
    f16 layer 2 in accumulation groups, DVE relu + 2 ACT relu+accum ops."""
    import concourse.mybir as mybir
    import concourse.tile as tile
    from concourse import bacc
    from contextlib import ExitStack

    dt = mybir.dt
    f32 = dt.float32
    split = mode == "f32r_split"
    mm_dt = {"f32r": dt.float32r, "f32r_split": dt.float32r, "f32": f32,
             "f16": dt.float16}[mode]

    nc = bacc.Bacc(
        "TRN2",
        target_bir_lowering=False,
        debug=False,
        enable_asserts=False,
        num_devices=1,
    )

    d_xt = nc.dram_tensor("d_xt", [IN + 1, R], mm_dt, kind="ExternalInput").ap()
    d_w1 = nc.dram_tensor("d_w1", [IN + 1, H], mm_dt, kind="ExternalInput").ap()
    d_w2 = nc.dram_tensor("d_w2", [H, H], mm_dt, kind="ExternalInput").ap()
    d_pb2 = nc.dram_tensor("d_pb2", [H], f32, kind="ExternalInput").ap()
    if split:
        d_w1l = nc.dram_tensor("d_w1l", [IN + 1, H], mm_dt, kind="ExternalInput").ap()
        d_w2l = nc.dram_tensor("d_w2l", [H, H], mm_dt, kind="ExternalInput").ap()
    d_s = nc.dram_tensor("d_s", [128, 2], f32, kind="ExternalOutput").ap()

    Relu = mybir.ActivationFunctionType.Relu
    X = mybir.AxisListType.X

    with tile.TileContext(nc) as tc, ExitStack() as ctx:
        cpool = ctx.enter_context(tc.tile_pool(name="cpool", bufs=1))
        xpool = ctx.enter_context(tc.tile_pool(name="xpool", bufs=xbufs))
        hpool = ctx.enter_context(tc.tile_pool(name="hpool", bufs=hbufs))
        spool = ctx.enter_context(tc.tile_pool(name="spool", bufs=2))
        ps1p = ctx.enter_context(tc.tile_pool(name="ps1p", bufs=2, space="PSUM"))
        ps2p = ctx.enter_context(tc.tile_pool(name="ps2p", bufs=2, space="PSUM"))

        w1_sb = cpool.tile([IN + 1, H], mm_dt, name="w1_sb")
        nc.sync.dma_start(w1_sb[:], d_w1)
        w2_sb = []
        for k in range(2):
            t = cpool.tile([128, H], mm_dt, name=f"w2_sb{k}")
            nc.sync.dma_start(t[:], d_w2[k * 128 : (k + 1) * 128, :])
            w2_sb.append(t)
        if split:
            w1l_sb = cpool.tile([IN + 1, H], mm_dt, name="w1l_sb")
            nc.sync.dma_start(w1l_sb[:], d_w1l)
            w2l_sb = []
            for k in range(2):
                t = cpool.tile([128, H], mm_dt, name=f"w2l_sb{k}")
                nc.sync.dma_start(t[:], d_w2l[k * 128 : (k + 1) * 128, :])
                w2l_sb.append(t)
        pb2_sb = cpool.tile([128, 2], f32, name="pb2_sb")
        nc.sync.dma_start(pb2_sb[:], d_pb2.rearrange("(m p) -> p m", p=128))

        acc = cpool.tile([128, 2, NBLK], f32, name="acc")

        for b in [b for _ in range(iters) for b in range(NBLK)]:
            xt = xpool.tile([IN + 1, BLK], mm_dt, name="xt", tag="xt")
            nc.sync.dma_start(xt[:], d_xt[:, b * BLK : (b + 1) * BLK])
            xr = xt[:]

            ps1 = ps1p.tile([128, 2, 512], f32, name="ps1", tag="ps1")
            for m in range(2):
                ms = slice(m * 128, (m + 1) * 128)
                nc.tensor.matmul(
                    ps1[:, m, 0:BLK], w1_sb[:, ms], xr,
                    start=True, stop=not split,
                )
                if split:
                    nc.tensor.matmul(
                        ps1[:, m, 0:BLK], w1l_sb[:, ms], xr,
                        start=False, stop=True,
                    )

            h1 = hpool.tile([128, 2, BLK], mm_dt, name="h1", tag="h1")
            nc.vector.tensor_scalar_max(h1[:], ps1[:, :, 0:BLK], 0.0)

            ps2 = ps2p.tile([128, 2, 512], f32, name="ps2", tag="ps2")
            for m in range(2):
                ms = slice(m * 128, (m + 1) * 128)
                mms = []
                for k in range(2):
                    mms.append((w2_sb[k][:, ms], h1[:, k, :]))
                    if split:
                        mms.append((w2l_sb[k][:, ms], h1[:, k, :]))
                for i, (lw, rr) in enumerate(mms):
                    nc.tensor.matmul(
                        ps2[:, m, 0:BLK], lw, rr,
                        start=(i == 0), stop=(i == len(mms) - 1),
                    )

            scr0 = spool.tile([128, BLK], f32, name="scr0", tag="scr0")
            nc.scalar.activation(
                scr0[:], ps2[:, 0, 0:BLK], Relu,
                bias=pb2_sb[:, 0:1],
                accum_out=acc[:, 0, b : b + 1],
            )
            scr1 = spool.tile([128, BLK], f32, name="scr1", tag="scr1")
            nc.scalar.activation(
                scr1[:], ps2[:, 1, 0:BLK], Relu,
                bias=pb2_sb[:, 1:2],
                accum_out=acc[:, 1, b : b + 1],
            )

        s_sb = cpool.tile([128, 2], f32, name="s_sb")
        nc.vector.reduce_sum(s_sb[:], acc[:], axis=X)
        nc.sync.dma_start(d_s, s_sb[:])

    nc.compile()
    return nc


def _hi_lo(w: np.ndarray):
    import ml_dtypes

    hi = np.asarray(w, dtype=ml_dtypes.bfloat16).astype(np.float32)
    lo = (w - hi).astype(np.float32)
    return hi, lo


def _diffuse_quant(W: np.ndarray, qdt) -> np.ndarray:
    """Error-diffusion quantization down the contraction axis: keeps
    per-column cumulative quantization error near zero so the (positive-mean)
    h1 stream doesn't see a systematic bias."""
    Wq = np.empty(W.shape, np.float32)
    carry = np.zeros(W.shape[1], np.float32)
    for k in range(W.shape[0]):
        t = W[k] + carry
        q = t.astype(qdt).astype(np.float32)
        carry = t - q
        Wq[k] = q
    return Wq


def _prep_in_maps(inputs: dict, mode: str):
    import ml_dtypes

    x = np.asarray(inputs["x"], dtype=np.float32)
    pw1 = np.asarray(inputs["pw1"], dtype=np.float32)
    pb1 = np.asarray(inputs["pb1"], dtype=np.float32)
    pw2 = np.asarray(inputs["pw2"], dtype=np.float32)
    pb2 = np.asarray(inputs["pb2"], dtype=np.float32)

    if mode in ("f16", "f32r", "f32r_split", "f32"):
        split = mode == "f32r_split"
        w1_aug = np.concatenate([pw1, pb1[None, :]], axis=0)  # [65, H]
        if split:
            w1h, w1l = _hi_lo(w1_aug)
            w2h, w2l = _hi_lo(pw2)
        else:
            w1h, w2h = w1_aug, pw2
        mm_np = np.float16 if mode == "f16" else np.float32
        w1h = w1h.astype(mm_np)
        w2h = w2h.astype(mm_np)
        in_maps = []
        for c in range(N_CORES):
            xt = np.empty((IN + 1, R), mm_np)
            xt[:IN] = x[c * R : (c + 1) * R].T.astype(mm_np)
            xt[IN] = 1.0
            m = {"d_xt": xt, "d_w1": w1h, "d_w2": w2h, "d_pb2": pb2}
            if split:
                m["d_w1l"] = w1l
                m["d_w2l"] = w2l
            in_maps.append(m)
        return in_maps

    fp8 = mode in ("v3fp8", "v4", "v5", "v6", "v7", "v8")
    if fp8:
        w2q = _diffuse_quant(pw2, ml_dtypes.float8_e4m3)
        w2 = np.ascontiguousarray(
            w2q.reshape(2, 128, H).transpose(1, 0, 2)
        ).astype(ml_dtypes.float8_e4m3)  # [k, pair, m]
    else:
        w2 = pw2.astype(np.float16)

    pw1h = pw1.astype(np.float16)
    common: dict
    if mode == "v4":
        b1m = np.zeros((128, 128), np.float16)
        b1m[64] = pb1[0:128].astype(np.float16)
        b1m[96] = pb1[128:256].astype(np.float16)
        b2m = np.stack([pb2[0:128], pb2[128:256]], axis=1).astype(np.float32)
        common = {"d_w1": pw1h, "d_w2": w2, "d_b1": b1m, "d_b2": b2m}
    elif mode == "v6":
        w1a = np.concatenate([pw1h, pb1[None, :].astype(np.float16)], axis=0)
        b2m = np.stack([pb2[0:128], pb2[128:256]], axis=1).astype(np.float32)
        common = {"d_w1": w1a, "d_w2": w2, "d_b2": b2m}
    elif mode in ("v7", "v8", "v9", "v10"):
        w1d = np.concatenate([pw1h, pw1h], axis=0)  # [128, 256]
        b = np.stack(
            [pb1[0:128], pb1[128:256], pb2[0:128], pb2[128:256]], axis=1
        ).astype(np.float32)
        common = {"d_w1": w1d, "d_w2": w2, "d_b": b}
    else:
        b = np.stack(
            [pb1[0:128], pb1[128:256], pb2[0:128], pb2[128:256]], axis=1
        ).astype(np.float32)  # [128, 4]
        common = {"d_w1": pw1h, "d_w2": w2, "d_b": b}

    in_maps = []
    for c in range(N_CORES):
        xc = x[c * R : (c + 1) * R].T.astype(np.float16)  # [64, R]
        if mode == "v6":
            xt = np.empty((IN + 1, R), np.float16)
            xt[:IN] = xc
            xt[IN] = 1.0
        elif mode in ("v7", "v8", "v10"):
            # interleave the pair's two 500-row blocks across partition
            # halves: [0:64] = even blocks, [64:128] = odd blocks.
            xr = xc.reshape(IN, NPAIR, 2, BLK)
            xt = np.concatenate(
                [
                    np.ascontiguousarray(xr[:, :, 0, :]).reshape(IN, R // 2),
                    np.ascontiguousarray(xr[:, :, 1, :]).reshape(IN, R // 2),
                ],
                axis=0,
            )  # [128, R//2]
        else:
            xt = np.ascontiguousarray(xc)
        in_maps.append({"d_xt": xt, **common})
    return in_maps


def _host_tail(S: np.ndarray, inputs: dict) -> np.ndarray:
    f = np.float64

    def g(name):
        return np.asarray(inputs[name], dtype=f)

    phi_sum = S @ g("pw3") + N * g("pb3")
    r = np.maximum(phi_sum @ g("rw1") + g("rb1"), 0.0)
    r = np.maximum(r @ g("rw2") + g("rb2"), 0.0)
    r = r @ g("rw3") + g("rb3")
    v = np.concatenate([r, g("x_static")])
    v = np.maximum(v @ g("w1") + g("b1"), 0.0)
    v = np.maximum(v @ g("w2") + g("b2"), 0.0)
    return (v @ g("w3") + g("b3")).astype(np.float32)


def _run(inputs: dict, trace: bool = False, mode: str | None = None):
    from concourse.bass_utils import run_bass_kernel_spmd

    mode = mode or MODE
    nc = _prog_cache.get(mode)
    if nc is None:
        nc = _build(mode)
        _prog_cache[mode] = nc

    if trace:
        try:
            import antenv.axon_hooks  # noqa: F401
        except ImportError:
            trace = False

    in_maps = _prep_in_maps(inputs, mode)
    res = run_bass_kernel_spmd(
        nc,
        in_maps,
        core_ids=list(range(N_CORES)),
        trace=trace,
    )

    S = np.zeros(H, np.float64)
    for rmap in res.results:
        s = rmap["d_s"].astype(np.float64)  # [128, 2]; channel = m*128 + p
        S += s.T.reshape(H)
    if mode in ("v4", "v5", "v6", "v7", "v8"):
        # the DVE path for h2-half1 accumulates sum(max(ps2, -b2)); the
        # + b2 * row-count shift is exact and lands here.
        S[128:256] += N * np.asarray(inputs["pb2"], np.float64)[128:256]
    out = _host_tail(S, inputs)
    return out, res


def kernel(**inputs) -> np.ndarray:
    out, _ = _run(inputs)
    return out



# revision 15
# speedup vs baseline: 1.5960x; 1.5960x over previous
"""Trainium2 Bass kernel for nn_DQN_34136400069239 (DeepSets-style pooling).

Math (reference):
    h1  = relu(x @ pw1 + pb1)          [N, H]
    h2  = relu(h1 @ pw2 + pb2)         [N, H]
    phi = h2 @ pw3 + pb3               [N, F]
    fp  = sum(phi, axis=0)             [F]
    ... tiny rho MLP + concat(x_static) + tiny 3-layer MLP -> [OUT]

The third phi layer is linear, so fp = (sum_n h2[n]) @ pw3 + N * pb3 and the
device only computes S = sum_n relu(h2[n]) in R^H.  Data-parallel over rows:
8 cores x 50000 rows, host sums the 8 partial S vectors and runs the tail.

Default mode "v3" (measured 136.5 us local-slope vs 176.1 us for the staged
f16 baseline; rel err 2.5e-4), per 1000-row pair of 500-row blocks:
  - All-f16 matmuls: 4x K=64 layer-1 mms and 8x K=128 layer-2 mms, the
    latter in 2-mm start/stop accumulation groups.  The dense grouped PE
    stream keeps the HAM activity monitor un-throttled at 2.4 GHz (~6000
    warm cycles/pair = ~2.5 us, the measured bound).  fp8 DoubleRow
    variants ("v7": 3260 cyc/pair) are ALGEBRAICALLY cheaper but their
    sparser single-mm pattern leaves the PE clock-gated cold at 1.2 GHz
    (~2.8 us) - warm f16 beats cold fp8.
  - PSUM is pair-level and half-major (ps1_h/ps2_h = [128, 2(block), 512]),
    so every vector-engine op covers one h-half of both blocks with a
    uniform per-partition f32 bias vector and accum_out keeps per-channel
    sums:
      DVE: (a) h1 = max(ps1+b1, 0) -> f16, tensor_scalar(add, max) per half
      ACT: (b) relu(ps2+b2) + fused row-sum accum_out per half
Other modes kept for comparison: f16/f32r/f32r_split = the original staged
baseline (f16: 176 us); v3fp8/v5/v6/v7/v8 = fp8-DoubleRow restructures
(v7 best at 141 us, rel err 3.4e-3; v4 is broken).  Numerical traps found
on the way: tensor_scalar accum_out reduces with op1 and applies scalar2
only ONCE (not per element), and plain fp8 rounding of W2 fails the 2e-2
gate (2.3e-2) while error diffusion down the contraction axis passes.
"""

import os

import numpy as np

# Problem constants (hardcoded; kernel.py must be self-contained).
N = 400000
IN, H, F, S_STATIC, OUT = 64, 256, 128, 16, 5
N_CORES = 8
R = N // N_CORES  # rows per core = 50000
BLK = 500  # matmul moving free dim
NBLK = R // BLK  # 100
NPAIR = NBLK // 2  # 50

MODE = os.environ.get("DQN_MODE", "v21")

_prog_cache: dict = {}


def _build(mode: str, iters: int = 1):
    if mode == "v4":
        return _build_v4(iters)
    if mode == "v5":
        return _build_v5(iters)
    if mode == "v6":
        return _build_v6(iters)
    if mode == "v7":
        return _build_v7(iters)
    if mode == "v8":
        return _build_v8(iters)
    if mode == "v10":
        return _build_v10(iters)
    if mode == "v11":
        return _build_v11(iters)
    if mode == "v12":
        return _build_v13(iters, mixed=True)
    if mode == "v13":
        return _build_v13(iters, mixed=False)
    if mode == "v16":
        return _build_v16(iters, dve_a1=False)
    if mode == "v17":
        return _build_v16(iters, dve_a1=True)
    if mode == "v18":
        return _build_v18(iters)
    if mode == "v19":
        return _build_v19(iters)
    if mode == "v20":
        return _build_v20(iters)
    if mode == "v21":
        return _build_v21(iters)
    if mode in ("f16", "f32r", "f32r_split", "f32"):
        return _build_base(mode, iters)
    import concourse.mybir as mybir
    import concourse.tile as tile
    from concourse import bacc
    from contextlib import ExitStack

    dt = mybir.dt
    f32 = dt.float32
    f16 = dt.float16
    fp8 = mode == "v3fp8"
    h1_dt = dt.float8e4 if fp8 else f16

    nc = bacc.Bacc(
        "TRN2",
        target_bir_lowering=False,
        debug=False,
        enable_asserts=False,
        num_devices=1,
    )

    d_xt = nc.dram_tensor("d_xt", [IN, R], f16, kind="ExternalInput").ap()
    d_w1 = nc.dram_tensor("d_w1", [IN, H], f16, kind="ExternalInput").ap()
    if fp8:
        # packed [k, pair, m]: W2p[k, i, m] = W2q[128*i + k, m]
        d_w2 = nc.dram_tensor("d_w2", [128, 2, H], dt.float8e4, kind="ExternalInput").ap()
    else:
        d_w2 = nc.dram_tensor("d_w2", [H, H], f16, kind="ExternalInput").ap()
    # f32 per-partition biases: cols = [b1_h0, b1_h1, b2_h0, b2_h1]
    d_b = nc.dram_tensor("d_b", [128, 4], f32, kind="ExternalInput").ap()
    d_s = nc.dram_tensor("d_s", [128, 2], f32, kind="ExternalOutput").ap()

    Relu = mybir.ActivationFunctionType.Relu
    Alu = mybir.AluOpType
    X = mybir.AxisListType.X

    with tile.TileContext(nc) as tc, ExitStack() as ctx:
        cpool = ctx.enter_context(tc.tile_pool(name="cpool", bufs=1))
        xpool = ctx.enter_context(tc.tile_pool(name="xpool", bufs=3))
        hpool = ctx.enter_context(tc.tile_pool(name="hpool", bufs=2))
        spool = ctx.enter_context(tc.tile_pool(name="spool", bufs=1))
        ps1p = ctx.enter_context(tc.tile_pool(name="ps1p", bufs=1, space="PSUM"))
        ps2p = ctx.enter_context(tc.tile_pool(name="ps2p", bufs=1, space="PSUM"))

        # Constants resident in SBUF.
        w1_sb = cpool.tile([IN, H], f16, name="w1_sb")
        nc.sync.dma_start(w1_sb[:], d_w1)
        if fp8:
            w2p_sb = cpool.tile([128, 2, H], dt.float8e4, name="w2p_sb")
            nc.sync.dma_start(w2p_sb[:], d_w2)
        else:
            w2_sb = []
            for k in range(2):
                t = cpool.tile([128, H], f16, name=f"w2_sb{k}")
                nc.sync.dma_start(t[:], d_w2[k * 128 : (k + 1) * 128, :])
                w2_sb.append(t)
        bv = cpool.tile([128, 4], f32, name="bv")
        nc.sync.dma_start(bv[:], d_b)

        # Per-pair accumulated row-sums of relu(h2), one column per pair.
        acc = [cpool.tile([128, NPAIR], f32, name=f"acc{m}") for m in range(2)]

        for pair in [p for _ in range(iters) for p in range(NPAIR)]:
            xt = xpool.tile([IN, 2 * BLK], f16, name="xt", tag="xt")
            nc.sync.dma_start(xt[:], d_xt[:, pair * 2 * BLK : (pair + 1) * 2 * BLK])

            ps1 = [
                ps1p.tile([128, 2, 512], f32, name=f"ps1_{m}", tag=f"ps1_{m}")
                for m in range(2)
            ]
            ps2 = [
                ps2p.tile([128, 2, 512], f32, name=f"ps2_{m}", tag=f"ps2_{m}")
                for m in range(2)
            ]

            # Layer 1: 4 K=64 matmuls into half-major pair psum.
            for j in range(2):
                xr = xt[:, j * BLK : (j + 1) * BLK]
                for m in range(2):
                    nc.tensor.matmul(
                        ps1[m][:, j, 0:BLK],
                        w1_sb[:, m * 128 : (m + 1) * 128],
                        xr,
                        start=True,
                        stop=True,
                    )

            # h1 = relu(ps1 + b1): one DVE op per half (uniform bias vector).
            h1 = hpool.tile([128, 2, 2, 512], h1_dt, name="h1", tag="h1")
            for m in range(2):
                nc.vector.tensor_scalar(
                    h1[:, m, :, 0:BLK],
                    ps1[m][:, :, 0:BLK],
                    bv[:, m : m + 1],
                    0.0,
                    op0=Alu.add,
                    op1=Alu.max,
                )

            # Layer 2 into pair-level psum.
            for j in range(2):
                if fp8:
                    for m in range(2):
                        nc.tensor.matmul(
                            ps2[m][:, j, 0:BLK],
                            w2p_sb[:, :, m * 128 : (m + 1) * 128],
                            h1[:, :, j, 0:BLK],
                            start=True,
                            stop=True,
                            perf_mode=mybir.MatmulPerfMode.DoubleRow,
                        )
                else:
                    for m in range(2):
                        for k in range(2):
                            nc.tensor.matmul(
                                ps2[m][:, j, 0:BLK],
                                w2_sb[k][:, m * 128 : (m + 1) * 128],
                                h1[:, k, j, 0:BLK],
                                start=(k == 0),
                                stop=(k == 1),
                            )

            # relu(ps2 + b2) with fused row-sum; channels preserved because
            # each op spans one half of both blocks.
            for m in range(2):
                scr = spool.tile([128, 2, 512], f16, name=f"scr{m}", tag=f"scr{m}")
                nc.scalar.activation(
                    scr[:, :, 0:BLK],
                    ps2[m][:, :, 0:BLK],
                    Relu,
                    bias=bv[:, 2 + m : 3 + m],
                    accum_out=acc[m][:, pair : pair + 1],
                )

        s_sb = cpool.tile([128, 2], f32, name="s_sb")
        for m in range(2):
            nc.vector.reduce_sum(s_sb[:, m : m + 1], acc[m][:], axis=X)
        nc.sync.dma_start(d_s, s_sb[:])

    nc.compile()
    return nc


def _build_v4(iters: int = 1):
    """ACT-centric fp8 variant.

    Empirical per-op costs (probe.py, chained, psum-f32 src):
      ACT  = ~383 + 0.25*FD ns   (f16 out; 4x-packed stream)
      DVE  = ~397 + 0.71*FD ns
    so ACT is the cheap drain and op count is what matters.  Per 1000-row
    pair: ONE ACT op does relu(ps1) for all four [half,block] layer-1 banks
    (FD=2000; b1 pre-added by K=1 ones-matmuls on PE strips 2-3, concurrent
    with the K=64 layer-1 matmuls on strips 0-1); layer-2 relu+accum runs
    half0 on ACT, half1 on DVE (bias as per-partition vector operands).
    Layer 2 is 2 fp8 DoubleRow matmuls per block (K_eff=256), weights-outer
    so LDWEIGHTS amortizes over the pair.
    """
    import concourse.mybir as mybir
    import concourse.tile as tile
    from concourse import bacc
    from contextlib import ExitStack

    dt = mybir.dt
    f32 = dt.float32
    f16 = dt.float16
    fp8 = dt.float8e4
    Relu = mybir.ActivationFunctionType.Relu
    Alu = mybir.AluOpType
    X = mybir.AxisListType.X

    nc = bacc.Bacc(
        "TRN2",
        target_bir_lowering=False,
        debug=False,
        enable_asserts=False,
        num_devices=1,
    )

    d_xt = nc.dram_tensor("d_xt", [IN, R], f16, kind="ExternalInput").ap()
    d_w1 = nc.dram_tensor("d_w1", [IN, H], f16, kind="ExternalInput").ap()
    d_w2 = nc.dram_tensor("d_w2", [128, 2, H], fp8, kind="ExternalInput").ap()
    # f16 b1 halves for the ones-matmuls, rows 64/96; f32 b2 via vector ops.
    d_b1 = nc.dram_tensor("d_b1", [128, 128], f16, kind="ExternalInput").ap()
    d_b2 = nc.dram_tensor("d_b2", [128, 2], f32, kind="ExternalInput").ap()
    d_s = nc.dram_tensor("d_s", [128, 2], f32, kind="ExternalOutput").ap()

    with tile.TileContext(nc) as tc, ExitStack() as ctx:
        cpool = ctx.enter_context(tc.tile_pool(name="cpool", bufs=1))
        xpool = ctx.enter_context(tc.tile_pool(name="xpool", bufs=3))
        hpool = ctx.enter_context(tc.tile_pool(name="hpool", bufs=2))
        spool = ctx.enter_context(tc.tile_pool(name="spool", bufs=1))
        ps1p = ctx.enter_context(tc.tile_pool(name="ps1p", bufs=1, space="PSUM"))
        ps2p = ctx.enter_context(tc.tile_pool(name="ps2p", bufs=1, space="PSUM"))

        w1_sb = cpool.tile([IN, H], f16, name="w1_sb")
        nc.sync.dma_start(w1_sb[:], d_w1)
        w2p_sb = cpool.tile([128, 2, H], fp8, name="w2p_sb")
        nc.sync.dma_start(w2p_sb[:], d_w2)
        b1_sb = cpool.tile([128, 128], f16, name="b1_sb")
        nc.sync.dma_start(b1_sb[:], d_b1)
        b2_sb = cpool.tile([128, 2], f32, name="b2_sb")
        nc.sync.dma_start(b2_sb[:], d_b2)
        nb2 = cpool.tile([128, 1], f32, name="nb2")
        nc.vector.tensor_scalar_mul(nb2[:], b2_sb[:, 1:2], -1.0)
        ones_sb = cpool.tile([128, BLK], f16, name="ones_sb")
        nc.vector.memset(ones_sb[:], 1.0)

        acc = [cpool.tile([128, NPAIR], f32, name=f"acc{m}") for m in range(2)]

        for pair in [p for _ in range(iters) for p in range(NPAIR)]:
            xt = xpool.tile([IN, 2 * BLK], f16, name="xt", tag="xt")
            nc.sync.dma_start(xt[:], d_xt[:, pair * 2 * BLK : (pair + 1) * 2 * BLK])

            # ps1: [half, block] banks, 4 banks, one tile per pair.
            ps1 = ps1p.tile([128, 2, 2, 512], f32, name="ps1", tag="ps1")
            for j in range(2):
                xr = xt[:, j * BLK : (j + 1) * BLK]
                for m in range(2):
                    strip = 64 if m == 0 else 96
                    nc.tensor.matmul(
                        ps1[:, m, j, 0:BLK],
                        b1_sb[strip : strip + 1, 0:128],
                        ones_sb[strip : strip + 1, 0:BLK],
                        start=True,
                        stop=False,
                        tile_position=(strip, 0),
                        skip_group_check=True,
                    )
                    nc.tensor.matmul(
                        ps1[:, m, j, 0:BLK],
                        w1_sb[:, m * 128 : (m + 1) * 128],
                        xr,
                        start=False,
                        stop=True,
                        skip_group_check=True,
                    )

            # (a): one ACT op drains all of ps1 -> packed fp8 h1.
            h1 = hpool.tile([128, 2, 2, 512], fp8, name="h1", tag="h1")
            nc.scalar.activation(h1[:, :, :, 0:BLK], ps1[:, :, :, 0:BLK], Relu)

            # Layer 2: DoubleRow, weights-outer so each half's LDWEIGHTS is
            # shared by both blocks of the pair.
            ps2 = [
                ps2p.tile([128, 2, 512], f32, name=f"ps2_{m}", tag=f"ps2_{m}")
                for m in range(2)
            ]
            for m in range(2):
                for j in range(2):
                    nc.tensor.matmul(
                        ps2[m][:, j, 0:BLK],
                        w2p_sb[:, :, m * 128 : (m + 1) * 128],
                        h1[:, :, j, 0:BLK],
                        start=True,
                        stop=True,
                        perf_mode=mybir.MatmulPerfMode.DoubleRow,
                    )

            # (b): relu(ps2 + b2) + per-channel row-sum; half0 on ACT,
            # half1 on DVE so the two drains run in parallel.
            scr0 = spool.tile([128, 2, 512], f16, name="scr0", tag="scr0")
            nc.scalar.activation(
                scr0[:, :, 0:BLK],
                ps2[0][:, :, 0:BLK],
                Relu,
                bias=b2_sb[:, 0:1],
                accum_out=acc[0][:, pair : pair + 1],
            )
            scr1 = spool.tile([128, 2, 512], f16, name="scr1", tag="scr1")
            nc.vector.tensor_scalar(
                scr1[:, :, 0:BLK],
                ps2[1][:, :, 0:BLK],
                nb2[:],
                None,
                op0=Alu.max,
                op1=Alu.add,
                accum_out=acc[1][:, pair : pair + 1],
            )

        s_sb = cpool.tile([128, 2], f32, name="s_sb")
        for m in range(2):
            nc.vector.reduce_sum(s_sb[:, m : m + 1], acc[m][:], axis=X)
        nc.sync.dma_start(d_s, s_sb[:])

    nc.compile()
    return nc


def _build_v5(iters: int = 1):
    """fp8 DoubleRow layer 2 with probe-informed engine split.

    Empirical per-op costs (probe.py, chained, psum-f32 src, FD=1000):
      ACT relu+bias(+accum) ~633 ns ;  DVE 2-op(+accum) ~1267 ns
    Per 1000-row pair (ops all pair-level, half-major so the per-partition
    bias vector is uniform within each op):
      ACT: (a)h0, (a)h1  relu(ps1+b1)->fp8 h1,  (b)h0 relu+accum  ~1.9 us
      DVE: (b)h1 relu+accum                                       ~1.3 us
      PE : 4x K=64 f16 layer-1 mm + 4x DoubleRow K_eff=256 layer-2 mm
           (weights-outer so each half's LDWEIGHTS covers both blocks)
    """
    import concourse.mybir as mybir
    import concourse.tile as tile
    from concourse import bacc
    from contextlib import ExitStack

    dt = mybir.dt
    f32 = dt.float32
    f16 = dt.float16
    fp8 = dt.float8e4
    Relu = mybir.ActivationFunctionType.Relu
    Alu = mybir.AluOpType
    X = mybir.AxisListType.X

    nc = bacc.Bacc(
        "TRN2",
        target_bir_lowering=False,
        debug=False,
        enable_asserts=False,
        num_devices=1,
    )

    d_xt = nc.dram_tensor("d_xt", [IN, R], f16, kind="ExternalInput").ap()
    d_w1 = nc.dram_tensor("d_w1", [IN, H], f16, kind="ExternalInput").ap()
    d_w2 = nc.dram_tensor("d_w2", [128, 2, H], fp8, kind="ExternalInput").ap()
    # f32 per-partition biases: cols = [b1_h0, b1_h1, b2_h0, b2_h1]
    d_b = nc.dram_tensor("d_b", [128, 4], f32, kind="ExternalInput").ap()
    d_s = nc.dram_tensor("d_s", [128, 2], f32, kind="ExternalOutput").ap()

    with tile.TileContext(nc) as tc, ExitStack() as ctx:
        cpool = ctx.enter_context(tc.tile_pool(name="cpool", bufs=1))
        xpool = ctx.enter_context(tc.tile_pool(name="xpool", bufs=3))
        hpool = ctx.enter_context(tc.tile_pool(name="hpool", bufs=2))
        spool = ctx.enter_context(tc.tile_pool(name="spool", bufs=1))
        ps1p = ctx.enter_context(tc.tile_pool(name="ps1p", bufs=1, space="PSUM"))
        ps2p = ctx.enter_context(tc.tile_pool(name="ps2p", bufs=1, space="PSUM"))

        w1_sb = cpool.tile([IN, H], f16, name="w1_sb")
        nc.sync.dma_start(w1_sb[:], d_w1)
        w2p_sb = cpool.tile([128, 2, H], fp8, name="w2p_sb")
        nc.sync.dma_start(w2p_sb[:], d_w2)
        bv = cpool.tile([128, 4], f32, name="bv")
        nc.sync.dma_start(bv[:], d_b)
        # negated b2_h1 for the DVE path: out = max(ps2, -b2) + b2, so the
        # accum reduce op (== op1) is add and accum_out is a true sum.
        nb2 = cpool.tile([128, 1], f32, name="nb2")
        nc.vector.tensor_scalar_mul(nb2[:], bv[:, 3:4], -1.0)

        acc = [cpool.tile([128, NPAIR], f32, name=f"acc{m}") for m in range(2)]

        for pair in [p for _ in range(iters) for p in range(NPAIR)]:
            xt = xpool.tile([IN, 2 * BLK], f16, name="xt", tag="xt")
            nc.sync.dma_start(xt[:], d_xt[:, pair * 2 * BLK : (pair + 1) * 2 * BLK])

            ps1 = [
                ps1p.tile([128, 2, 512], f32, name=f"ps1_{m}", tag=f"ps1_{m}")
                for m in range(2)
            ]
            for j in range(2):
                xr = xt[:, j * BLK : (j + 1) * BLK]
                for m in range(2):
                    nc.tensor.matmul(
                        ps1[m][:, j, 0:BLK],
                        w1_sb[:, m * 128 : (m + 1) * 128],
                        xr,
                        start=True,
                        stop=True,
                    )

            # (a): h1 = relu(ps1 + b1) -> packed fp8, one ACT op per half.
            h1 = hpool.tile([128, 2, 2, 512], fp8, name="h1", tag="h1")
            for m in range(2):
                nc.scalar.activation(
                    h1[:, m, :, 0:BLK],
                    ps1[m][:, :, 0:BLK],
                    Relu,
                    bias=bv[:, m : m + 1],
                )

            # Layer 2: DoubleRow, weights-outer so each half's LDWEIGHTS is
            # shared by both blocks of the pair.
            ps2 = [
                ps2p.tile([128, 2, 512], f32, name=f"ps2_{m}", tag=f"ps2_{m}")
                for m in range(2)
            ]
            for m in range(2):
                for j in range(2):
                    nc.tensor.matmul(
                        ps2[m][:, j, 0:BLK],
                        w2p_sb[:, :, m * 128 : (m + 1) * 128],
                        h1[:, :, j, 0:BLK],
                        start=True,
                        stop=True,
                        perf_mode=mybir.MatmulPerfMode.DoubleRow,
                    )

            # (b): relu(ps2 + b2) + per-channel row-sum; half0 on ACT,
            # half1 on DVE so the two drains run in parallel.
            scr0 = spool.tile([128, 2, 512], f16, name="scr0", tag="scr0")
            nc.scalar.activation(
                scr0[:, :, 0:BLK],
                ps2[0][:, :, 0:BLK],
                Relu,
                bias=bv[:, 2:3],
                accum_out=acc[0][:, pair : pair + 1],
            )
            scr1 = spool.tile([128, 2, 512], f16, name="scr1", tag="scr1")
            nc.vector.tensor_scalar(
                scr1[:, :, 0:BLK],
                ps2[1][:, :, 0:BLK],
                nb2[:],
                None,
                op0=Alu.max,
                op1=Alu.add,
                accum_out=acc[1][:, pair : pair + 1],
            )

        s_sb = cpool.tile([128, 2], f32, name="s_sb")
        for m in range(2):
            nc.vector.reduce_sum(s_sb[:, m : m + 1], acc[m][:], axis=X)
        nc.sync.dma_start(d_s, s_sb[:])

    nc.compile()
    return nc


def _build_v6(iters: int = 1):
    """Like v5 but layer-1 bias rides in the matmul contraction (K=65
    ones-row, as in the original baseline), so layer-1 relu needs no bias
    and collapses to ONE ACT op per pair over a single 4-bank psum tile
    with slot index (2*block + half):
      ACT: (a) relu(ps1)->fp8 h1 FD=2000,  (b)h0 relu+bias+accum FD=1000
      DVE: (b)h1 add-bias+max+accum FD=1000
      PE : 4x K=65 f16 layer-1 mm + 4x DoubleRow layer-2 mm per pair
    """
    import concourse.mybir as mybir
    import concourse.tile as tile
    from concourse import bacc
    from contextlib import ExitStack

    dt = mybir.dt
    f32 = dt.float32
    f16 = dt.float16
    fp8 = dt.float8e4
    Relu = mybir.ActivationFunctionType.Relu
    Alu = mybir.AluOpType
    X = mybir.AxisListType.X

    nc = bacc.Bacc(
        "TRN2",
        target_bir_lowering=False,
        debug=False,
        enable_asserts=False,
        num_devices=1,
    )

    d_xt = nc.dram_tensor("d_xt", [IN + 1, R], f16, kind="ExternalInput").ap()
    d_w1 = nc.dram_tensor("d_w1", [IN + 1, H], f16, kind="ExternalInput").ap()
    d_w2 = nc.dram_tensor("d_w2", [128, 2, H], fp8, kind="ExternalInput").ap()
    d_b2 = nc.dram_tensor("d_b2", [128, 2], f32, kind="ExternalInput").ap()
    d_s = nc.dram_tensor("d_s", [128, 2], f32, kind="ExternalOutput").ap()

    with tile.TileContext(nc) as tc, ExitStack() as ctx:
        cpool = ctx.enter_context(tc.tile_pool(name="cpool", bufs=1))
        xpool = ctx.enter_context(tc.tile_pool(name="xpool", bufs=3))
        hpool = ctx.enter_context(tc.tile_pool(name="hpool", bufs=2))
        spool = ctx.enter_context(tc.tile_pool(name="spool", bufs=1))
        ps1p = ctx.enter_context(tc.tile_pool(name="ps1p", bufs=1, space="PSUM"))
        ps2p = ctx.enter_context(tc.tile_pool(name="ps2p", bufs=1, space="PSUM"))

        w1_sb = cpool.tile([IN + 1, H], f16, name="w1_sb")
        nc.sync.dma_start(w1_sb[:], d_w1)
        w2p_sb = cpool.tile([128, 2, H], fp8, name="w2p_sb")
        nc.sync.dma_start(w2p_sb[:], d_w2)
        b2_sb = cpool.tile([128, 2], f32, name="b2_sb")
        nc.sync.dma_start(b2_sb[:], d_b2)
        # negated b2_h1 for the DVE path: out = max(ps2, -b2) + b2, so the
        # accum reduce op (== op1) is add and accum_out is a true sum.
        nb2 = cpool.tile([128, 1], f32, name="nb2")
        nc.vector.tensor_scalar_mul(nb2[:], b2_sb[:, 1:2], -1.0)

        acc = [cpool.tile([128, NPAIR], f32, name=f"acc{m}") for m in range(2)]

        for pair in [p for _ in range(iters) for p in range(NPAIR)]:
            xt = xpool.tile([IN + 1, 2 * BLK], f16, name="xt", tag="xt")
            nc.sync.dma_start(xt[:], d_xt[:, pair * 2 * BLK : (pair + 1) * 2 * BLK])

            # ps1 slots: index 2*block + half.
            ps1 = ps1p.tile([128, 4, 512], f32, name="ps1", tag="ps1")
            for j in range(2):
                xr = xt[:, j * BLK : (j + 1) * BLK]
                for m in range(2):
                    nc.tensor.matmul(
                        ps1[:, 2 * j + m, 0:BLK],
                        w1_sb[:, m * 128 : (m + 1) * 128],
                        xr,
                        start=True,
                        stop=True,
                    )

            # (a): ONE ACT op drains all four layer-1 banks -> packed fp8.
            h1 = hpool.tile([128, 4, 512], fp8, name="h1", tag="h1")
            nc.scalar.activation(h1[:, :, 0:BLK], ps1[:, :, 0:BLK], Relu)

            # Layer 2: DoubleRow, weights-outer; rhs pair-dim = half
            # (slots 2j..2j+1 are the two halves of block j).
            ps2 = [
                ps2p.tile([128, 2, 512], f32, name=f"ps2_{m}", tag=f"ps2_{m}")
                for m in range(2)
            ]
            for m in range(2):
                for j in range(2):
                    nc.tensor.matmul(
                        ps2[m][:, j, 0:BLK],
                        w2p_sb[:, :, m * 128 : (m + 1) * 128],
                        h1[:, 2 * j : 2 * j + 2, 0:BLK],
                        start=True,
                        stop=True,
                        perf_mode=mybir.MatmulPerfMode.DoubleRow,
                    )

            # (b): relu(ps2 + b2) + per-channel row-sum; half0 on ACT,
            # half1 on DVE.
            scr0 = spool.tile([128, 2, 512], f16, name="scr0", tag="scr0")
            nc.scalar.activation(
                scr0[:, :, 0:BLK],
                ps2[0][:, :, 0:BLK],
                Relu,
                bias=b2_sb[:, 0:1],
                accum_out=acc[0][:, pair : pair + 1],
            )
            scr1 = spool.tile([128, 2, 512], f16, name="scr1", tag="scr1")
            nc.vector.tensor_scalar(
                scr1[:, :, 0:BLK],
                ps2[1][:, :, 0:BLK],
                nb2[:],
                None,
                op0=Alu.max,
                op1=Alu.add,
                accum_out=acc[1][:, pair : pair + 1],
            )

        s_sb = cpool.tile([128, 2], f32, name="s_sb")
        for m in range(2):
            nc.vector.reduce_sum(s_sb[:, m : m + 1], acc[m][:], axis=X)
        nc.sync.dma_start(d_s, s_sb[:])

    nc.compile()
    return nc


def _build_v8(iters: int = 1):
    """v5 with strip-concurrent layer 1.

    Layer-1 matmuls are K=64 and use only PE row strips 0-1, so two of them
    run CONCURRENTLY when placed at tile_position (0,0) and (64,0): the host
    interleaves the pair's two 500-row blocks across partition halves
    (block A features in partitions 0-63, block B in 64-127) and W1 is
    duplicated across both halves.  Each pass computes one h-half of block A
    alongside the other h-half of block B, so layer 1 costs ~1000 PE cycles
    per pair instead of 2000.  Everything else is v5: fp8 DoubleRow layer 2,
    (a) = 2 pair-level ACT relu+bias ops, (b) = ACT half0 / DVE half1 (DVE
    accumulates sum(max(ps2,-b2)); + b2*N lands in the host tail).
    """
    import concourse.mybir as mybir
    import concourse.tile as tile
    from concourse import bacc
    from contextlib import ExitStack

    dt = mybir.dt
    f32 = dt.float32
    f16 = dt.float16
    fp8 = dt.float8e4
    Relu = mybir.ActivationFunctionType.Relu
    Alu = mybir.AluOpType
    X = mybir.AxisListType.X

    R2 = R // 2  # columns of the interleaved x layout

    nc = bacc.Bacc(
        "TRN2",
        target_bir_lowering=False,
        debug=False,
        enable_asserts=False,
        num_devices=1,
    )

    d_xt = nc.dram_tensor("d_xt", [128, R2], f16, kind="ExternalInput").ap()
    d_w1 = nc.dram_tensor("d_w1", [128, H], f16, kind="ExternalInput").ap()
    d_w2 = nc.dram_tensor("d_w2", [128, 2, H], fp8, kind="ExternalInput").ap()
    d_b = nc.dram_tensor("d_b", [128, 4], f32, kind="ExternalInput").ap()
    d_s = nc.dram_tensor("d_s", [128, 2], f32, kind="ExternalOutput").ap()

    with tile.TileContext(nc) as tc, ExitStack() as ctx:
        cpool = ctx.enter_context(tc.tile_pool(name="cpool", bufs=1))
        xpool = ctx.enter_context(tc.tile_pool(name="xpool", bufs=3))
        hpool = ctx.enter_context(tc.tile_pool(name="hpool", bufs=3))
        spool = ctx.enter_context(tc.tile_pool(name="spool", bufs=1))
        ps1p = ctx.enter_context(tc.tile_pool(name="ps1p", bufs=1, space="PSUM"))
        ps2p = ctx.enter_context(tc.tile_pool(name="ps2p", bufs=1, space="PSUM"))

        # W1 duplicated across both partition halves: [128, 256] f16.
        w1_sb = cpool.tile([128, H], f16, name="w1_sb")
        nc.sync.dma_start(w1_sb[:], d_w1)
        w2p_sb = cpool.tile([128, 2, H], fp8, name="w2p_sb")
        nc.sync.dma_start(w2p_sb[:], d_w2)
        bv = cpool.tile([128, 4], f32, name="bv")
        nc.sync.dma_start(bv[:], d_b)
        nb2 = cpool.tile([128, 1], f32, name="nb2")
        nc.vector.tensor_scalar_mul(nb2[:], bv[:, 3:4], -1.0)

        acc = [cpool.tile([128, NPAIR], f32, name=f"acc{m}") for m in range(2)]

        for pair in [p for _ in range(iters) for p in range(NPAIR)]:
            # [128, 500]: partitions 0-63 = block A features, 64-127 = block B.
            xt = xpool.tile([128, BLK], f16, name="xt", tag="xt")
            nc.sync.dma_start(xt[:], d_xt[:, pair * BLK : (pair + 1) * BLK])

            ps1 = [
                ps1p.tile([128, 2, 512], f32, name=f"ps1_{m}", tag=f"ps1_{m}")
                for m in range(2)
            ]
            # Pass 1: A-h0 on strips 0-1 || B-h1 on strips 2-3.
            nc.tensor.matmul(
                ps1[0][:, 0, 0:BLK], w1_sb[0:64, 0:128], xt[0:64, 0:BLK],
                start=True, stop=True,
            )
            nc.tensor.matmul(
                ps1[1][:, 1, 0:BLK], w1_sb[64:128, 128:256], xt[64:128, 0:BLK],
                start=True, stop=True,
            )
            # Pass 2: A-h1 || B-h0.
            nc.tensor.matmul(
                ps1[1][:, 0, 0:BLK], w1_sb[0:64, 128:256], xt[0:64, 0:BLK],
                start=True, stop=True,
            )
            nc.tensor.matmul(
                ps1[0][:, 1, 0:BLK], w1_sb[64:128, 0:128], xt[64:128, 0:BLK],
                start=True, stop=True,
            )

            # (a): h1 = relu(ps1 + b1) -> packed fp8, one ACT op per half.
            h1 = hpool.tile([128, 2, 2, 512], fp8, name="h1", tag="h1")
            for m in range(2):
                nc.scalar.activation(
                    h1[:, m, :, 0:BLK],
                    ps1[m][:, :, 0:BLK],
                    Relu,
                    bias=bv[:, m : m + 1],
                )

            ps2 = [
                ps2p.tile([128, 2, 512], f32, name=f"ps2_{m}", tag=f"ps2_{m}")
                for m in range(2)
            ]
            for j in range(2):
                for m in range(2):
                    nc.tensor.matmul(
                        ps2[m][:, j, 0:BLK],
                        w2p_sb[:, :, m * 128 : (m + 1) * 128],
                        h1[:, :, j, 0:BLK],
                        start=True,
                        stop=True,
                        perf_mode=mybir.MatmulPerfMode.DoubleRow,
                    )
            # keep-warm blips: tiny matmuls into psum padding keep PE
            # activity in every HAM window (~50 ns each, no readers).
            nc.tensor.matmul(
                ps1[0][0:8, 0, 500:512], w1_sb[0:1, 0:8], w1_sb[0:1, 0:12],
                start=True, stop=True,
            )
            nc.tensor.matmul(
                ps1[1][0:8, 0, 500:512], w1_sb[0:1, 0:8], w1_sb[0:1, 0:12],
                start=True, stop=True,
            )

            scr0 = spool.tile([128, 2, 512], f16, name="scr0", tag="scr0")
            nc.scalar.activation(
                scr0[:, :, 0:BLK],
                ps2[0][:, :, 0:BLK],
                Relu,
                bias=bv[:, 2:3],
                accum_out=acc[0][:, pair : pair + 1],
            )
            scr1 = spool.tile([128, 2, 512], f16, name="scr1", tag="scr1")
            nc.vector.tensor_scalar(
                scr1[:, :, 0:BLK],
                ps2[1][:, :, 0:BLK],
                nb2[:],
                None,
                op0=Alu.max,
                op1=Alu.add,
                accum_out=acc[1][:, pair : pair + 1],
            )

        s_sb = cpool.tile([128, 2], f32, name="s_sb")
        for m in range(2):
            nc.vector.reduce_sum(s_sb[:, m : m + 1], acc[m][:], axis=X)
        nc.sync.dma_start(d_s, s_sb[:])

    nc.compile()
    return nc


def _build_v7(iters: int = 1):
    """v5 with strip-concurrent layer 1.

    Layer-1 matmuls are K=64 and use only PE row strips 0-1, so two of them
    run CONCURRENTLY when placed at tile_position (0,0) and (64,0): the host
    interleaves the pair's two 500-row blocks across partition halves
    (block A features in partitions 0-63, block B in 64-127) and W1 is
    duplicated across both halves.  Each pass computes one h-half of block A
    alongside the other h-half of block B, so layer 1 costs ~1000 PE cycles
    per pair instead of 2000.  Everything else is v5: fp8 DoubleRow layer 2,
    (a) = 2 pair-level ACT relu+bias ops, (b) = ACT half0 / DVE half1 (DVE
    accumulates sum(max(ps2,-b2)); + b2*N lands in the host tail).
    """
    import concourse.mybir as mybir
    import concourse.tile as tile
    from concourse import bacc
    from contextlib import ExitStack

    dt = mybir.dt
    f32 = dt.float32
    f16 = dt.float16
    fp8 = dt.float8e4
    Relu = mybir.ActivationFunctionType.Relu
    Alu = mybir.AluOpType
    X = mybir.AxisListType.X

    R2 = R // 2  # columns of the interleaved x layout

    nc = bacc.Bacc(
        "TRN2",
        target_bir_lowering=False,
        debug=False,
        enable_asserts=False,
        num_devices=1,
    )

    d_xt = nc.dram_tensor("d_xt", [128, R2], f16, kind="ExternalInput").ap()
    d_w1 = nc.dram_tensor("d_w1", [128, H], f16, kind="ExternalInput").ap()
    d_w2 = nc.dram_tensor("d_w2", [128, 2, H], fp8, kind="ExternalInput").ap()
    d_b = nc.dram_tensor("d_b", [128, 4], f32, kind="ExternalInput").ap()
    d_s = nc.dram_tensor("d_s", [128, 2], f32, kind="ExternalOutput").ap()

    with tile.TileContext(nc) as tc, ExitStack() as ctx:
        cpool = ctx.enter_context(tc.tile_pool(name="cpool", bufs=1))
        xpool = ctx.enter_context(tc.tile_pool(name="xpool", bufs=3))
        hpool = ctx.enter_context(tc.tile_pool(name="hpool", bufs=2))
        spool = ctx.enter_context(tc.tile_pool(name="spool", bufs=1))
        ps1p = ctx.enter_context(tc.tile_pool(name="ps1p", bufs=1, space="PSUM"))
        ps2p = ctx.enter_context(tc.tile_pool(name="ps2p", bufs=1, space="PSUM"))

        # W1 duplicated across both partition halves: [128, 256] f16.
        w1_sb = cpool.tile([128, H], f16, name="w1_sb")
        nc.sync.dma_start(w1_sb[:], d_w1)
        w2p_sb = cpool.tile([128, 2, H], fp8, name="w2p_sb")
        nc.sync.dma_start(w2p_sb[:], d_w2)
        bv = cpool.tile([128, 4], f32, name="bv")
        nc.sync.dma_start(bv[:], d_b)
        nb2 = cpool.tile([128, 1], f32, name="nb2")
        nc.vector.tensor_scalar_mul(nb2[:], bv[:, 3:4], -1.0)

        acc = [cpool.tile([128, NPAIR], f32, name=f"acc{m}") for m in range(2)]

        for pair in [p for _ in range(iters) for p in range(NPAIR)]:
            # [128, 500]: partitions 0-63 = block A features, 64-127 = block B.
            xt = xpool.tile([128, BLK], f16, name="xt", tag="xt")
            nc.sync.dma_start(xt[:], d_xt[:, pair * BLK : (pair + 1) * BLK])

            ps1 = [
                ps1p.tile([128, 2, 512], f32, name=f"ps1_{m}", tag=f"ps1_{m}")
                for m in range(2)
            ]
            # Pass 1: A-h0 on strips 0-1 || B-h1 on strips 2-3.
            nc.tensor.matmul(
                ps1[0][:, 0, 0:BLK], w1_sb[0:64, 0:128], xt[0:64, 0:BLK],
                start=True, stop=True,
            )
            nc.tensor.matmul(
                ps1[1][:, 1, 0:BLK], w1_sb[64:128, 128:256], xt[64:128, 0:BLK],
                start=True, stop=True,
            )
            # Pass 2: A-h1 || B-h0.
            nc.tensor.matmul(
                ps1[1][:, 0, 0:BLK], w1_sb[0:64, 128:256], xt[0:64, 0:BLK],
                start=True, stop=True,
            )
            nc.tensor.matmul(
                ps1[0][:, 1, 0:BLK], w1_sb[64:128, 0:128], xt[64:128, 0:BLK],
                start=True, stop=True,
            )

            # (a): h1 = relu(ps1 + b1) -> packed fp8, one ACT op per half.
            h1 = hpool.tile([128, 2, 2, 512], fp8, name="h1", tag="h1")
            for m in range(2):
                nc.scalar.activation(
                    h1[:, m, :, 0:BLK],
                    ps1[m][:, :, 0:BLK],
                    Relu,
                    bias=bv[:, m : m + 1],
                )

            ps2 = [
                ps2p.tile([128, 2, 512], f32, name=f"ps2_{m}", tag=f"ps2_{m}")
                for m in range(2)
            ]
            for m in range(2):
                for j in range(2):
                    nc.tensor.matmul(
                        ps2[m][:, j, 0:BLK],
                        w2p_sb[:, :, m * 128 : (m + 1) * 128],
                        h1[:, :, j, 0:BLK],
                        start=True,
                        stop=True,
                        perf_mode=mybir.MatmulPerfMode.DoubleRow,
                    )

            scr0 = spool.tile([128, 2, 512], f16, name="scr0", tag="scr0")
            nc.scalar.activation(
                scr0[:, :, 0:BLK],
                ps2[0][:, :, 0:BLK],
                Relu,
                bias=bv[:, 2:3],
                accum_out=acc[0][:, pair : pair + 1],
            )
            scr1 = spool.tile([128, 2, 512], f16, name="scr1", tag="scr1")
            nc.vector.tensor_scalar(
                scr1[:, :, 0:BLK],
                ps2[1][:, :, 0:BLK],
                nb2[:],
                None,
                op0=Alu.max,
                op1=Alu.add,
                accum_out=acc[1][:, pair : pair + 1],
            )

        s_sb = cpool.tile([128, 2], f32, name="s_sb")
        for m in range(2):
            nc.vector.reduce_sum(s_sb[:, m : m + 1], acc[m][:], axis=X)
        nc.sync.dma_start(d_s, s_sb[:])

    nc.compile()
    return nc


def _build_v10(iters: int = 1):
    """v3 (warm all-f16) + strip-concurrent layer 1: same grouped f16
    layer-2 stream that keeps the PE un-throttled at 2.4 GHz, but layer-1's
    four K=64 matmuls run as two concurrent pairs at tile_position (0,0) /
    (64,0) with the pair's blocks interleaved across partition halves."""
    import concourse.mybir as mybir
    import concourse.tile as tile
    from concourse import bacc
    from contextlib import ExitStack

    dt = mybir.dt
    f32 = dt.float32
    f16 = dt.float16
    Relu = mybir.ActivationFunctionType.Relu
    Alu = mybir.AluOpType
    X = mybir.AxisListType.X
    R2 = R // 2

    nc = bacc.Bacc("TRN2", target_bir_lowering=False, debug=False,
                   enable_asserts=False, num_devices=1)

    d_xt = nc.dram_tensor("d_xt", [128, R2], f16, kind="ExternalInput").ap()
    d_w1 = nc.dram_tensor("d_w1", [128, H], f16, kind="ExternalInput").ap()
    d_w2 = nc.dram_tensor("d_w2", [H, H], f16, kind="ExternalInput").ap()
    d_b = nc.dram_tensor("d_b", [128, 4], f32, kind="ExternalInput").ap()
    d_s = nc.dram_tensor("d_s", [128, 2], f32, kind="ExternalOutput").ap()

    with tile.TileContext(nc) as tc, ExitStack() as ctx:
        cpool = ctx.enter_context(tc.tile_pool(name="cpool", bufs=1))
        xpool = ctx.enter_context(tc.tile_pool(name="xpool", bufs=3))
        hpool = ctx.enter_context(tc.tile_pool(name="hpool", bufs=2))
        spool = ctx.enter_context(tc.tile_pool(name="spool", bufs=1))
        ps1p = ctx.enter_context(tc.tile_pool(name="ps1p", bufs=1, space="PSUM"))
        ps2p = ctx.enter_context(tc.tile_pool(name="ps2p", bufs=1, space="PSUM"))

        w1_sb = cpool.tile([128, H], f16, name="w1_sb")
        nc.sync.dma_start(w1_sb[:], d_w1)
        w2_sb = []
        for k in range(2):
            t = cpool.tile([128, H], f16, name=f"w2_sb{k}")
            nc.sync.dma_start(t[:], d_w2[k * 128 : (k + 1) * 128, :])
            w2_sb.append(t)
        bv = cpool.tile([128, 4], f32, name="bv")
        nc.sync.dma_start(bv[:], d_b)

        acc = [cpool.tile([128, NPAIR], f32, name=f"acc{m}") for m in range(2)]

        for pair in [p for _ in range(iters) for p in range(NPAIR)]:
            xt = xpool.tile([128, BLK], f16, name="xt", tag="xt")
            nc.sync.dma_start(xt[:], d_xt[:, pair * BLK : (pair + 1) * BLK])

            ps1 = [
                ps1p.tile([128, 2, 512], f32, name=f"ps1_{m}", tag=f"ps1_{m}")
                for m in range(2)
            ]
            ps2 = [
                ps2p.tile([128, 2, 512], f32, name=f"ps2_{m}", tag=f"ps2_{m}")
                for m in range(2)
            ]

            # Layer 1: two concurrent passes; pass 1 completes ps1[0].
            nc.tensor.matmul(ps1[0][:, 0, 0:BLK], w1_sb[0:64, 0:128],
                             xt[0:64, 0:BLK], start=True, stop=True)
            nc.tensor.matmul(ps1[0][:, 1, 0:BLK], w1_sb[64:128, 0:128],
                             xt[64:128, 0:BLK], start=True, stop=True)
            nc.tensor.matmul(ps1[1][:, 0, 0:BLK], w1_sb[0:64, 128:256],
                             xt[0:64, 0:BLK], start=True, stop=True)
            nc.tensor.matmul(ps1[1][:, 1, 0:BLK], w1_sb[64:128, 128:256],
                             xt[64:128, 0:BLK], start=True, stop=True)

            h1 = hpool.tile([128, 2, 2, 512], f16, name="h1", tag="h1")
            for m in range(2):
                nc.vector.tensor_scalar(
                    h1[:, m, :, 0:BLK], ps1[m][:, :, 0:BLK],
                    bv[:, m : m + 1], 0.0, op0=Alu.add, op1=Alu.max,
                )

            for j in range(2):
                for m in range(2):
                    for k in range(2):
                        nc.tensor.matmul(
                            ps2[m][:, j, 0:BLK],
                            w2_sb[k][:, m * 128 : (m + 1) * 128],
                            h1[:, k, j, 0:BLK],
                            start=(k == 0),
                            stop=(k == 1),
                        )

            for m in range(2):
                scr = spool.tile([128, 2, 512], f16, name=f"scr{m}", tag=f"scr{m}")
                nc.scalar.activation(
                    scr[:, :, 0:BLK], ps2[m][:, :, 0:BLK], Relu,
                    bias=bv[:, 2 + m : 3 + m],
                    accum_out=acc[m][:, pair : pair + 1],
                )

        s_sb = cpool.tile([128, 2], f32, name="s_sb")
        for m in range(2):
            nc.vector.reduce_sum(s_sb[:, m : m + 1], acc[m][:], axis=X)
        nc.sync.dma_start(d_s, s_sb[:])

    nc.compile()
    return nc


def _build_v11(iters: int = 1):
    """v3's all-f16 matmul structure with strip-concurrent layer 1 and an
    ACT-heavy drain split.

    Per 1000-row pair the PE does 2 concurrent-strip layer-1 passes
    (1000 cyc) + 8 grouped K=128 layer-2 mms (4000 cyc) = 5000 cyc
    (~2083 ns warm).  Drains are rebalanced so neither ACT nor DVE exceeds
    that: ACT gets (a)h0, (a)h1 (relu+bias -> f16 h1) and (b)h0
    (relu+bias+accum), ~1899 ns; DVE gets (b)h1 via the max(ps,-b2)+accum
    trick, ~1107 ns (the +b2*N shift lands in the host tail).
    """
    import concourse.mybir as mybir
    import concourse.tile as tile
    from concourse import bacc
    from contextlib import ExitStack

    dt = mybir.dt
    f32 = dt.float32
    f16 = dt.float16
    Relu = mybir.ActivationFunctionType.Relu
    Alu = mybir.AluOpType
    X = mybir.AxisListType.X
    R2 = R // 2

    nc = bacc.Bacc("TRN2", target_bir_lowering=False, debug=False,
                   enable_asserts=False, num_devices=1)

    d_xt = nc.dram_tensor("d_xt", [128, R2], f16, kind="ExternalInput").ap()
    d_w1 = nc.dram_tensor("d_w1", [128, H], f16, kind="ExternalInput").ap()
    d_w2 = nc.dram_tensor("d_w2", [H, H], f16, kind="ExternalInput").ap()
    d_b = nc.dram_tensor("d_b", [128, 4], f32, kind="ExternalInput").ap()
    d_s = nc.dram_tensor("d_s", [128, 2], f32, kind="ExternalOutput").ap()

    with tile.TileContext(nc) as tc, ExitStack() as ctx:
        cpool = ctx.enter_context(tc.tile_pool(name="cpool", bufs=1))
        xpool = ctx.enter_context(tc.tile_pool(name="xpool", bufs=3))
        hpool = ctx.enter_context(tc.tile_pool(name="hpool", bufs=2))
        spool = ctx.enter_context(tc.tile_pool(name="spool", bufs=1))
        ps1p = ctx.enter_context(tc.tile_pool(name="ps1p", bufs=1, space="PSUM"))
        ps2p = ctx.enter_context(tc.tile_pool(name="ps2p", bufs=1, space="PSUM"))

        w1_sb = cpool.tile([128, H], f16, name="w1_sb")
        nc.sync.dma_start(w1_sb[:], d_w1)
        w2_sb = []
        for k in range(2):
            tw = cpool.tile([128, H], f16, name=f"w2_sb{k}")
            nc.sync.dma_start(tw[:], d_w2[k * 128 : (k + 1) * 128, :])
            w2_sb.append(tw)
        bv = cpool.tile([128, 4], f32, name="bv")
        nc.sync.dma_start(bv[:], d_b)
        nb2 = cpool.tile([128, 1], f32, name="nb2")
        nc.vector.tensor_scalar_mul(nb2[:], bv[:, 3:4], -1.0)

        acc = [cpool.tile([128, NPAIR], f32, name=f"acc{m}") for m in range(2)]

        for pair in [p for _ in range(iters) for p in range(NPAIR)]:
            xt = xpool.tile([128, BLK], f16, name="xt", tag="xt")
            nc.sync.dma_start(xt[:], d_xt[:, pair * BLK : (pair + 1) * BLK])

            ps1 = [
                ps1p.tile([128, 2, 512], f32, name=f"ps1_{m}", tag=f"ps1_{m}")
                for m in range(2)
            ]
            # Pass 1: A-h0 on strips 0-1 || B-h1 on strips 2-3.
            nc.tensor.matmul(
                ps1[0][:, 0, 0:BLK], w1_sb[0:64, 0:128], xt[0:64, 0:BLK],
                start=True, stop=True,
            )
            nc.tensor.matmul(
                ps1[1][:, 1, 0:BLK], w1_sb[64:128, 128:256], xt[64:128, 0:BLK],
                start=True, stop=True,
            )
            # Pass 2: A-h1 || B-h0.
            nc.tensor.matmul(
                ps1[1][:, 0, 0:BLK], w1_sb[0:64, 128:256], xt[0:64, 0:BLK],
                start=True, stop=True,
            )
            nc.tensor.matmul(
                ps1[0][:, 1, 0:BLK], w1_sb[64:128, 0:128], xt[64:128, 0:BLK],
                start=True, stop=True,
            )

            # (a): h1 = relu(ps1 + b1) -> f16, one ACT op per half.
            h1 = hpool.tile([128, 2, 2, 512], f16, name="h1", tag="h1")
            for m in range(2):
                nc.scalar.activation(
                    h1[:, m, :, 0:BLK],
                    ps1[m][:, :, 0:BLK],
                    Relu,
                    bias=bv[:, m : m + 1],
                )

            # Layer 2: f16 accumulation groups, m-half outer so ps2[0]
            # completes early for the ACT (b) drain.
            ps2 = [
                ps2p.tile([128, 2, 512], f32, name=f"ps2_{m}", tag=f"ps2_{m}")
                for m in range(2)
            ]
            for m in range(2):
                for j in range(2):
                    for k in range(2):
                        nc.tensor.matmul(
                            ps2[m][:, j, 0:BLK],
                            w2_sb[k][:, m * 128 : (m + 1) * 128],
                            h1[:, k, j, 0:BLK],
                            start=(k == 0),
                            stop=(k == 1),
                        )

            scr0 = spool.tile([128, 2, 512], f16, name="scr0", tag="scr0")
            nc.scalar.activation(
                scr0[:, :, 0:BLK],
                ps2[0][:, :, 0:BLK],
                Relu,
                bias=bv[:, 2:3],
                accum_out=acc[0][:, pair : pair + 1],
            )
            scr1 = spool.tile([128, 2, 512], f16, name="scr1", tag="scr1")
            nc.vector.tensor_scalar(
                scr1[:, :, 0:BLK],
                ps2[1][:, :, 0:BLK],
                nb2[:],
                None,
                op0=Alu.max,
                op1=Alu.add,
                accum_out=acc[1][:, pair : pair + 1],
            )

        s_sb = cpool.tile([128, 2], f32, name="s_sb")
        for m in range(2):
            nc.vector.reduce_sum(s_sb[:, m : m + 1], acc[m][:], axis=X)
        nc.sync.dma_start(d_s, s_sb[:])

    nc.compile()
    return nc


def _build_v13(iters: int = 1, mixed: bool = False):
    """fp8 layer 2 with a single-op layer-1 drain via b1 channel pairing.

    The 256 h1 channels are permuted on the host so that the two channels
    living in the same partition (one per h-half psum bank) have adjacent
    sorted b1 values; the (a) drain then uses ONE ACT op over the whole
    4-bank ps1 tile with the pair-mean bias (|err| ~ half the adjacent-b1
    gap, ~5e-4).  W1 columns / W2 rows are permuted to match, so everything
    else is exact.

    Per 1000-row pair:
      PE : 2 concurrent-strip f16 layer-1 passes (1000 cyc) +
           mixed=False: 4 fp8 DoubleRow K_eff=256 layer-2 mms (~2260 cyc)
           mixed=True : 2 DoubleRow (m0) + 4 plain-fp8 K=128 mms (m1)
                        (~3130 cyc)
      ACT: (a) relu(ps1+bpair) -> fp8 h1, FD=2000 (~883 ns)
           (b)h0 relu+bias+accum, FD=1000 (~633 ns)
      DVE: (b)h1 max(ps2,-b2)+accum, FD=1000 (~1107 ns)
    """
    import concourse.mybir as mybir
    import concourse.tile as tile
    from concourse import bacc
    from contextlib import ExitStack

    dt = mybir.dt
    f32 = dt.float32
    f16 = dt.float16
    fp8 = dt.float8e4
    Relu = mybir.ActivationFunctionType.Relu
    Alu = mybir.AluOpType
    X = mybir.AxisListType.X
    R2 = R // 2

    nc = bacc.Bacc("TRN2", target_bir_lowering=False, debug=False,
                   enable_asserts=False, num_devices=1)

    d_xt = nc.dram_tensor("d_xt", [128, R2], f16, kind="ExternalInput").ap()
    d_w1 = nc.dram_tensor("d_w1", [128, H], f16, kind="ExternalInput").ap()
    d_w2 = nc.dram_tensor("d_w2", [128, 2, H], fp8, kind="ExternalInput").ap()
    d_b = nc.dram_tensor("d_b", [128, 4], f32, kind="ExternalInput").ap()
    d_s = nc.dram_tensor("d_s", [128, 2], f32, kind="ExternalOutput").ap()

    with tile.TileContext(nc) as tc, ExitStack() as ctx:
        cpool = ctx.enter_context(tc.tile_pool(name="cpool", bufs=1))
        xpool = ctx.enter_context(tc.tile_pool(name="xpool", bufs=3))
        hpool = ctx.enter_context(tc.tile_pool(name="hpool", bufs=2))
        spool = ctx.enter_context(tc.tile_pool(name="spool", bufs=1))
        ps1p = ctx.enter_context(tc.tile_pool(name="ps1p", bufs=1, space="PSUM"))
        ps2p = ctx.enter_context(tc.tile_pool(name="ps2p", bufs=1, space="PSUM"))

        w1_sb = cpool.tile([128, H], f16, name="w1_sb")
        nc.sync.dma_start(w1_sb[:], d_w1)
        w2p_sb = cpool.tile([128, 2, H], fp8, name="w2p_sb")
        nc.sync.dma_start(w2p_sb[:], d_w2)
        bv = cpool.tile([128, 4], f32, name="bv")
        nc.sync.dma_start(bv[:], d_b)
        nb2 = cpool.tile([128, 1], f32, name="nb2")
        nc.vector.tensor_scalar_mul(nb2[:], bv[:, 3:4], -1.0)

        acc = [cpool.tile([128, NPAIR], f32, name=f"acc{m}") for m in range(2)]

        for pair in [p for _ in range(iters) for p in range(NPAIR)]:
            xt = xpool.tile([128, BLK], f16, name="xt", tag="xt")
            nc.sync.dma_start(xt[:], d_xt[:, pair * BLK : (pair + 1) * BLK])

            # ps1: one 4-bank tile [128, m, j, 512].
            ps1 = ps1p.tile([128, 2, 2, 512], f32, name="ps1", tag="ps1")
            # Pass 1: A-h0 on strips 0-1 || B-h1 on strips 2-3.
            nc.tensor.matmul(
                ps1[:, 0, 0, 0:BLK], w1_sb[0:64, 0:128], xt[0:64, 0:BLK],
                start=True, stop=True,
            )
            nc.tensor.matmul(
                ps1[:, 1, 1, 0:BLK], w1_sb[64:128, 128:256], xt[64:128, 0:BLK],
                start=True, stop=True,
            )
            # Pass 2: A-h1 || B-h0.
            nc.tensor.matmul(
                ps1[:, 1, 0, 0:BLK], w1_sb[0:64, 128:256], xt[0:64, 0:BLK],
                start=True, stop=True,
            )
            nc.tensor.matmul(
                ps1[:, 0, 1, 0:BLK], w1_sb[64:128, 0:128], xt[64:128, 0:BLK],
                start=True, stop=True,
            )

            # (a): ONE ACT op drains the whole ps1 tile -> packed fp8 h1.
            h1 = hpool.tile([128, 2, 2, 512], fp8, name="h1", tag="h1")
            nc.scalar.activation(
                h1[:, :, :, 0:BLK],
                ps1[:, :, :, 0:BLK],
                Relu,
                bias=bv[:, 0:1],
            )

            # Layer 2: m0 via DoubleRow; m1 via DoubleRow (full) or plain
            # fp8 K=128 groups (mixed).  m0 first so ACT's (b) tile is
            # ready early.
            ps2 = [
                ps2p.tile([128, 2, 512], f32, name=f"ps2_{m}", tag=f"ps2_{m}")
                for m in range(2)
            ]
            for j in range(2):
                nc.tensor.matmul(
                    ps2[0][:, j, 0:BLK],
                    w2p_sb[:, :, 0:128],
                    h1[:, :, j, 0:BLK],
                    start=True,
                    stop=True,
                    perf_mode=mybir.MatmulPerfMode.DoubleRow,
                )
            if mixed:
                for j in range(2):
                    for k in range(2):
                        nc.tensor.matmul(
                            ps2[1][:, j, 0:BLK],
                            w2p_sb[:, k, 128:256],
                            h1[:, k, j, 0:BLK],
                            start=(k == 0),
                            stop=(k == 1),
                        )
            else:
                for j in range(2):
                    nc.tensor.matmul(
                        ps2[1][:, j, 0:BLK],
                        w2p_sb[:, :, 128:256],
                        h1[:, :, j, 0:BLK],
                        start=True,
                        stop=True,
                        perf_mode=mybir.MatmulPerfMode.DoubleRow,
                    )

            scr0 = spool.tile([128, 2, 512], f16, name="scr0", tag="scr0")
            nc.scalar.activation(
                scr0[:, :, 0:BLK],
                ps2[0][:, :, 0:BLK],
                Relu,
                bias=bv[:, 2:3],
                accum_out=acc[0][:, pair : pair + 1],
            )
            scr1 = spool.tile([128, 2, 512], f16, name="scr1", tag="scr1")
            nc.vector.tensor_scalar(
                scr1[:, :, 0:BLK],
                ps2[1][:, :, 0:BLK],
                nb2[:],
                None,
                op0=Alu.max,
                op1=Alu.add,
                accum_out=acc[1][:, pair : pair + 1],
            )

        s_sb = cpool.tile([128, 2], f32, name="s_sb")
        for m in range(2):
            nc.vector.reduce_sum(s_sb[:, m : m + 1], acc[m][:], axis=X)
        nc.sync.dma_start(d_s, s_sb[:])

    nc.compile()
    return nc


def _build_v16(iters: int = 1, dve_a1: bool = False):
    """v7 (strip-concurrent f16 L1 + fp8 DoubleRow L2) with fp8 junk-scratch
    outputs on the (b) drains (halves the drain engines' write cost; the
    accum_out column is what matters and stays f32).

    dve_a1=True additionally moves (a)h1 to the DVE (2-op add-bias/max,
    fp8 out, exact) for a balanced 2+2 drain split:
      ACT: (a)h0, (b)h0   DVE: (a)h1, (b)h1
    """
    import concourse.mybir as mybir
    import concourse.tile as tile
    from concourse import bacc
    from contextlib import ExitStack

    dt = mybir.dt
    f32 = dt.float32
    f16 = dt.float16
    fp8 = dt.float8e4
    Relu = mybir.ActivationFunctionType.Relu
    Alu = mybir.AluOpType
    X = mybir.AxisListType.X
    R2 = R // 2

    nc = bacc.Bacc("TRN2", target_bir_lowering=False, debug=False,
                   enable_asserts=False, num_devices=1)

    d_xt = nc.dram_tensor("d_xt", [128, R2], f16, kind="ExternalInput").ap()
    d_w1 = nc.dram_tensor("d_w1", [128, H], f16, kind="ExternalInput").ap()
    d_w2 = nc.dram_tensor("d_w2", [128, 2, H], fp8, kind="ExternalInput").ap()
    d_b = nc.dram_tensor("d_b", [128, 4], f32, kind="ExternalInput").ap()
    d_s = nc.dram_tensor("d_s", [128, 2], f32, kind="ExternalOutput").ap()

    with tile.TileContext(nc) as tc, ExitStack() as ctx:
        cpool = ctx.enter_context(tc.tile_pool(name="cpool", bufs=1))
        xpool = ctx.enter_context(tc.tile_pool(name="xpool", bufs=3))
        hpool = ctx.enter_context(tc.tile_pool(name="hpool", bufs=2))
        spool = ctx.enter_context(tc.tile_pool(name="spool", bufs=1))
        ps1p = ctx.enter_context(tc.tile_pool(name="ps1p", bufs=1, space="PSUM"))
        ps2p = ctx.enter_context(tc.tile_pool(name="ps2p", bufs=1, space="PSUM"))

        w1_sb = cpool.tile([128, H], f16, name="w1_sb")
        nc.sync.dma_start(w1_sb[:], d_w1)
        w2p_sb = cpool.tile([128, 2, H], fp8, name="w2p_sb")
        nc.sync.dma_start(w2p_sb[:], d_w2)
        bv = cpool.tile([128, 4], f32, name="bv")
        nc.sync.dma_start(bv[:], d_b)
        nb2 = cpool.tile([128, 1], f32, name="nb2")
        nc.vector.tensor_scalar_mul(nb2[:], bv[:, 3:4], -1.0)

        acc = [cpool.tile([128, NPAIR], f32, name=f"acc{m}") for m in range(2)]

        for pair in [p for _ in range(iters) for p in range(NPAIR)]:
            xt = xpool.tile([128, BLK], f16, name="xt", tag="xt")
            nc.sync.dma_start(xt[:], d_xt[:, pair * BLK : (pair + 1) * BLK])

            ps1 = [
                ps1p.tile([128, 2, 512], f32, name=f"ps1_{m}", tag=f"ps1_{m}")
                for m in range(2)
            ]
            nc.tensor.matmul(
                ps1[0][:, 0, 0:BLK], w1_sb[0:64, 0:128], xt[0:64, 0:BLK],
                start=True, stop=True,
            )
            nc.tensor.matmul(
                ps1[1][:, 1, 0:BLK], w1_sb[64:128, 128:256], xt[64:128, 0:BLK],
                start=True, stop=True,
            )
            nc.tensor.matmul(
                ps1[1][:, 0, 0:BLK], w1_sb[0:64, 128:256], xt[0:64, 0:BLK],
                start=True, stop=True,
            )
            nc.tensor.matmul(
                ps1[0][:, 1, 0:BLK], w1_sb[64:128, 0:128], xt[64:128, 0:BLK],
                start=True, stop=True,
            )

            # (a): h1 = relu(ps1 + b1) -> packed fp8.
            h1 = hpool.tile([128, 2, 2, 512], fp8, name="h1", tag="h1")
            nc.scalar.activation(
                h1[:, 0, :, 0:BLK], ps1[0][:, :, 0:BLK], Relu,
                bias=bv[:, 0:1],
            )
            if dve_a1:
                nc.vector.tensor_scalar(
                    h1[:, 1, :, 0:BLK], ps1[1][:, :, 0:BLK],
                    bv[:, 1:2], 0.0, op0=Alu.add, op1=Alu.max,
                )
            else:
                nc.scalar.activation(
                    h1[:, 1, :, 0:BLK], ps1[1][:, :, 0:BLK], Relu,
                    bias=bv[:, 1:2],
                )

            ps2 = [
                ps2p.tile([128, 2, 512], f32, name=f"ps2_{m}", tag=f"ps2_{m}")
                for m in range(2)
            ]
            for m in range(2):
                for j in range(2):
                    nc.tensor.matmul(
                        ps2[m][:, j, 0:BLK],
                        w2p_sb[:, :, m * 128 : (m + 1) * 128],
                        h1[:, :, j, 0:BLK],
                        start=True,
                        stop=True,
                        perf_mode=mybir.MatmulPerfMode.DoubleRow,
                    )

            scr0 = spool.tile([128, 2, 512], fp8, name="scr0", tag="scr0")
            nc.scalar.activation(
                scr0[:, :, 0:BLK],
                ps2[0][:, :, 0:BLK],
                Relu,
                bias=bv[:, 2:3],
                accum_out=acc[0][:, pair : pair + 1],
            )
            scr1 = spool.tile([128, 2, 512], fp8, name="scr1", tag="scr1")
            nc.vector.tensor_scalar(
                scr1[:, :, 0:BLK],
                ps2[1][:, :, 0:BLK],
                nb2[:],
                None,
                op0=Alu.max,
                op1=Alu.add,
                accum_out=acc[1][:, pair : pair + 1],
            )

        s_sb = cpool.tile([128, 2], f32, name="s_sb")
        for m in range(2):
            nc.vector.reduce_sum(s_sb[:, m : m + 1], acc[m][:], axis=X)
        nc.sync.dma_start(d_s, s_sb[:])

    nc.compile()
    return nc


def _build_v19(iters: int = 1):
    """v17 with PSUM bank cycling: layer 2 writes back into the SAME psum
    tiles layer 1 used (write-after-read), so one pair only holds 4 banks
    and bufs=2 gives true cross-pair double buffering.  This removes the
    L1 -> (a) -> L2 -> (b) serialization that cost ~760 ns/pair in the
    split-psum layout (skel decomposition: full 2816 vs max-component 2057).

    Engines per 1000-row pair:
      PE : 4 strip f16 L1 mms + 4 fp8 DoubleRow L2 mms
      ACT: (a)h0 relu+bias->fp8, (b)h0 relu+bias+accum (fp8 scr)
      DVE: (a)h1 add/max->fp8,   (b)h1 max/add+accum (fp8 scr)
    """
    import concourse.mybir as mybir
    import concourse.tile as tile
    from concourse import bacc
    from contextlib import ExitStack

    dt = mybir.dt
    f32 = dt.float32
    f16 = dt.float16
    fp8 = dt.float8e4
    Relu = mybir.ActivationFunctionType.Relu
    Alu = mybir.AluOpType
    X = mybir.AxisListType.X
    R2 = R // 2

    nc = bacc.Bacc("TRN2", target_bir_lowering=False, debug=False,
                   enable_asserts=False, num_devices=1)

    d_xt = nc.dram_tensor("d_xt", [128, R2], f16, kind="ExternalInput").ap()
    d_w1 = nc.dram_tensor("d_w1", [128, H], f16, kind="ExternalInput").ap()
    d_w2 = nc.dram_tensor("d_w2", [128, 2, H], fp8, kind="ExternalInput").ap()
    d_b = nc.dram_tensor("d_b", [128, 4], f32, kind="ExternalInput").ap()
    d_s = nc.dram_tensor("d_s", [128, 2], f32, kind="ExternalOutput").ap()

    with tile.TileContext(nc) as tc, ExitStack() as ctx:
        cpool = ctx.enter_context(tc.tile_pool(name="cpool", bufs=1))
        xpool = ctx.enter_context(tc.tile_pool(name="xpool", bufs=3))
        hpool = ctx.enter_context(tc.tile_pool(name="hpool", bufs=2))
        spool = ctx.enter_context(tc.tile_pool(name="spool", bufs=1))
        psp = ctx.enter_context(tc.tile_pool(name="psp", bufs=2, space="PSUM"))

        w1_sb = cpool.tile([128, H], f16, name="w1_sb")
        nc.sync.dma_start(w1_sb[:], d_w1)
        w2p_sb = cpool.tile([128, 2, H], fp8, name="w2p_sb")
        nc.sync.dma_start(w2p_sb[:], d_w2)
        bv = cpool.tile([128, 4], f32, name="bv")
        nc.sync.dma_start(bv[:], d_b)
        nb2 = cpool.tile([128, 1], f32, name="nb2")
        nc.vector.tensor_scalar_mul(nb2[:], bv[:, 3:4], -1.0)

        acc = [cpool.tile([128, NPAIR], f32, name=f"acc{m}") for m in range(2)]

        for pair in [p for _ in range(iters) for p in range(NPAIR)]:
            xt = xpool.tile([128, BLK], f16, name="xt", tag="xt")
            nc.sync.dma_start(xt[:], d_xt[:, pair * BLK : (pair + 1) * BLK])

            ps = [
                psp.tile([128, 2, 512], f32, name=f"ps_{m}", tag=f"ps_{m}")
                for m in range(2)
            ]
            nc.tensor.matmul(
                ps[0][:, 0, 0:BLK], w1_sb[0:64, 0:128], xt[0:64, 0:BLK],
                start=True, stop=True,
            )
            nc.tensor.matmul(
                ps[1][:, 1, 0:BLK], w1_sb[64:128, 128:256], xt[64:128, 0:BLK],
                start=True, stop=True,
            )
            nc.tensor.matmul(
                ps[1][:, 0, 0:BLK], w1_sb[0:64, 128:256], xt[0:64, 0:BLK],
                start=True, stop=True,
            )
            nc.tensor.matmul(
                ps[0][:, 1, 0:BLK], w1_sb[64:128, 0:128], xt[64:128, 0:BLK],
                start=True, stop=True,
            )

            # (a): h1 = relu(ps + b1) -> packed fp8; h0 on ACT, h1 on DVE.
            h1 = hpool.tile([128, 2, 2, 512], fp8, name="h1", tag="h1")
            nc.scalar.activation(
                h1[:, 0, :, 0:BLK], ps[0][:, :, 0:BLK], Relu,
                bias=bv[:, 0:1],
            )
            nc.vector.tensor_scalar(
                h1[:, 1, :, 0:BLK], ps[1][:, :, 0:BLK],
                bv[:, 1:2], 0.0, op0=Alu.add, op1=Alu.max,
            )

            # Layer 2 writes back into the same psum tiles (bank cycling).
            for m in range(2):
                for j in range(2):
                    nc.tensor.matmul(
                        ps[m][:, j, 0:BLK],
                        w2p_sb[:, :, m * 128 : (m + 1) * 128],
                        h1[:, :, j, 0:BLK],
                        start=True,
                        stop=True,
                        perf_mode=mybir.MatmulPerfMode.DoubleRow,
                    )

            scr0 = spool.tile([128, 2, 512], fp8, name="scr0", tag="scr0")
            nc.scalar.activation(
                scr0[:, :, 0:BLK],
                ps[0][:, :, 0:BLK],
                Relu,
                bias=bv[:, 2:3],
                accum_out=acc[0][:, pair : pair + 1],
            )
            scr1 = spool.tile([128, 2, 512], fp8, name="scr1", tag="scr1")
            nc.vector.tensor_scalar(
                scr1[:, :, 0:BLK],
                ps[1][:, :, 0:BLK],
                nb2[:],
                None,
                op0=Alu.max,
                op1=Alu.add,
                accum_out=acc[1][:, pair : pair + 1],
            )

        s_sb = cpool.tile([128, 2], f32, name="s_sb")
        for m in range(2):
            nc.vector.reduce_sum(s_sb[:, m : m + 1], acc[m][:], axis=X)
        nc.sync.dma_start(d_s, s_sb[:])

    nc.compile()
    return nc


def _build_v21(iters: int = 1):
    """v19 with deeper SBUF buffering (xpool 4, hpool 3, spool 2)."""
    import concourse.mybir as mybir
    import concourse.tile as tile
    from concourse import bacc
    from contextlib import ExitStack

    dt = mybir.dt
    f32 = dt.float32
    f16 = dt.float16
    fp8 = dt.float8e4
    Relu = mybir.ActivationFunctionType.Relu
    Alu = mybir.AluOpType
    X = mybir.AxisListType.X
    R2 = R // 2

    nc = bacc.Bacc("TRN2", target_bir_lowering=False, debug=False,
                   enable_asserts=False, num_devices=1)

    d_xt = nc.dram_tensor("d_xt", [128, R2], f16, kind="ExternalInput").ap()
    d_w1 = nc.dram_tensor("d_w1", [128, H], f16, kind="ExternalInput").ap()
    d_w2 = nc.dram_tensor("d_w2", [128, 2, H], fp8, kind="ExternalInput").ap()
    d_b = nc.dram_tensor("d_b", [128, 4], f32, kind="ExternalInput").ap()
    d_s = nc.dram_tensor("d_s", [128, 2], f32, kind="ExternalOutput").ap()

    with tile.TileContext(nc) as tc, ExitStack() as ctx:
        cpool = ctx.enter_context(tc.tile_pool(name="cpool", bufs=1))
        xpool = ctx.enter_context(tc.tile_pool(name="xpool", bufs=4))
        hpool = ctx.enter_context(tc.tile_pool(name="hpool", bufs=3))
        spool = ctx.enter_context(tc.tile_pool(name="spool", bufs=2))
        psp = ctx.enter_context(tc.tile_pool(name="psp", bufs=2, space="PSUM"))

        w1_sb = cpool.tile([128, H], f16, name="w1_sb")
        nc.sync.dma_start(w1_sb[:], d_w1)
        w2p_sb = cpool.tile([128, 2, H], fp8, name="w2p_sb")
        nc.sync.dma_start(w2p_sb[:], d_w2)
        bv = cpool.tile([128, 4], f32, name="bv")
        nc.sync.dma_start(bv[:], d_b)
        nb2 = cpool.tile([128, 1], f32, name="nb2")
        nc.vector.tensor_scalar_mul(nb2[:], bv[:, 3:4], -1.0)

        acc = [cpool.tile([128, NPAIR], f32, name=f"acc{m}") for m in range(2)]

        for pair in [p for _ in range(iters) for p in range(NPAIR)]:
            xt = xpool.tile([128, BLK], f16, name="xt", tag="xt")
            nc.sync.dma_start(xt[:], d_xt[:, pair * BLK : (pair + 1) * BLK])

            ps = [
                psp.tile([128, 2, 512], f32, name=f"ps_{m}", tag=f"ps_{m}")
                for m in range(2)
            ]
            nc.tensor.matmul(
                ps[0][:, 0, 0:BLK], w1_sb[0:64, 0:128], xt[0:64, 0:BLK],
                start=True, stop=True,
            )
            nc.tensor.matmul(
                ps[1][:, 1, 0:BLK], w1_sb[64:128, 128:256], xt[64:128, 0:BLK],
                start=True, stop=True,
            )
            nc.tensor.matmul(
                ps[1][:, 0, 0:BLK], w1_sb[0:64, 128:256], xt[0:64, 0:BLK],
                start=True, stop=True,
            )
            nc.tensor.matmul(
                ps[0][:, 1, 0:BLK], w1_sb[64:128, 0:128], xt[64:128, 0:BLK],
                start=True, stop=True,
            )

            # (a): h1 = relu(ps + b1) -> packed fp8; h0 on ACT, h1 on DVE.
            h1 = hpool.tile([128, 2, 2, 512], fp8, name="h1", tag="h1")
            nc.scalar.activation(
                h1[:, 0, :, 0:BLK], ps[0][:, :, 0:BLK], Relu,
                bias=bv[:, 0:1],
            )
            nc.vector.tensor_scalar(
                h1[:, 1, :, 0:BLK], ps[1][:, :, 0:BLK],
                bv[:, 1:2], 0.0, op0=Alu.add, op1=Alu.max,
            )

            # Layer 2 writes back into the same psum tiles (bank cycling).
            for m in range(2):
                for j in range(2):
                    nc.tensor.matmul(
                        ps[m][:, j, 0:BLK],
                        w2p_sb[:, :, m * 128 : (m + 1) * 128],
                        h1[:, :, j, 0:BLK],
                        start=True,
                        stop=True,
                        perf_mode=mybir.MatmulPerfMode.DoubleRow,
                    )

            scr0 = spool.tile([128, 2, 512], fp8, name="scr0", tag="scr0")
            nc.scalar.activation(
                scr0[:, :, 0:BLK],
                ps[0][:, :, 0:BLK],
                Relu,
                bias=bv[:, 2:3],
                accum_out=acc[0][:, pair : pair + 1],
            )
            scr1 = spool.tile([128, 2, 512], fp8, name="scr1", tag="scr1")
            nc.vector.tensor_scalar(
                scr1[:, :, 0:BLK],
                ps[1][:, :, 0:BLK],
                nb2[:],
                None,
                op0=Alu.max,
                op1=Alu.add,
                accum_out=acc[1][:, pair : pair + 1],
            )

        s_sb = cpool.tile([128, 2], f32, name="s_sb")
        for m in range(2):
            nc.vector.reduce_sum(s_sb[:, m : m + 1], acc[m][:], axis=X)
        nc.sync.dma_start(d_s, s_sb[:])

    nc.compile()
    return nc



def _build_v20(iters: int = 1):
    """v19 bank-cycling with all-f16 layer 2 (h1 f16, grouped K=128 mms)."""
    import concourse.mybir as mybir
    import concourse.tile as tile
    from concourse import bacc
    from contextlib import ExitStack

    dt = mybir.dt
    f32 = dt.float32
    f16 = dt.float16
    fp8 = dt.float8e4
    Relu = mybir.ActivationFunctionType.Relu
    Alu = mybir.AluOpType
    X = mybir.AxisListType.X
    R2 = R // 2

    nc = bacc.Bacc("TRN2", target_bir_lowering=False, debug=False,
                   enable_asserts=False, num_devices=1)

    d_xt = nc.dram_tensor("d_xt", [128, R2], f16, kind="ExternalInput").ap()
    d_w1 = nc.dram_tensor("d_w1", [128, H], f16, kind="ExternalInput").ap()
    d_w2 = nc.dram_tensor("d_w2", [H, H], f16, kind="ExternalInput").ap()
    d_b = nc.dram_tensor("d_b", [128, 4], f32, kind="ExternalInput").ap()
    d_s = nc.dram_tensor("d_s", [128, 2], f32, kind="ExternalOutput").ap()

    with tile.TileContext(nc) as tc, ExitStack() as ctx:
        cpool = ctx.enter_context(tc.tile_pool(name="cpool", bufs=1))
        xpool = ctx.enter_context(tc.tile_pool(name="xpool", bufs=3))
        hpool = ctx.enter_context(tc.tile_pool(name="hpool", bufs=2))
        spool = ctx.enter_context(tc.tile_pool(name="spool", bufs=1))
        psp = ctx.enter_context(tc.tile_pool(name="psp", bufs=2, space="PSUM"))

        w1_sb = cpool.tile([128, H], f16, name="w1_sb")
        nc.sync.dma_start(w1_sb[:], d_w1)
        w2_sb = []
        for k in range(2):
            tw = cpool.tile([128, H], f16, name=f"w2_sb{k}")
            nc.sync.dma_start(tw[:], d_w2[k * 128 : (k + 1) * 128, :])
            w2_sb.append(tw)
        bv = cpool.tile([128, 4], f32, name="bv")
        nc.sync.dma_start(bv[:], d_b)
        nb2 = cpool.tile([128, 1], f32, name="nb2")
        nc.vector.tensor_scalar_mul(nb2[:], bv[:, 3:4], -1.0)

        acc = [cpool.tile([128, NPAIR], f32, name=f"acc{m}") for m in range(2)]

        for pair in [p for _ in range(iters) for p in range(NPAIR)]:
            xt = xpool.tile([128, BLK], f16, name="xt", tag="xt")
            nc.sync.dma_start(xt[:], d_xt[:, pair * BLK : (pair + 1) * BLK])

            ps = [
                psp.tile([128, 2, 512], f32, name=f"ps_{m}", tag=f"ps_{m}")
                for m in range(2)
            ]
            nc.tensor.matmul(
                ps[0][:, 0, 0:BLK], w1_sb[0:64, 0:128], xt[0:64, 0:BLK],
                start=True, stop=True,
            )
            nc.tensor.matmul(
                ps[1][:, 1, 0:BLK], w1_sb[64:128, 128:256], xt[64:128, 0:BLK],
                start=True, stop=True,
            )
            nc.tensor.matmul(
                ps[1][:, 0, 0:BLK], w1_sb[0:64, 128:256], xt[0:64, 0:BLK],
                start=True, stop=True,
            )
            nc.tensor.matmul(
                ps[0][:, 1, 0:BLK], w1_sb[64:128, 0:128], xt[64:128, 0:BLK],
                start=True, stop=True,
            )

            # (a): h1 = relu(ps + b1) -> packed fp8; h0 on ACT, h1 on DVE.
            h1 = hpool.tile([128, 2, 2, 512], f16, name="h1", tag="h1")
            nc.scalar.activation(
                h1[:, 0, :, 0:BLK], ps[0][:, :, 0:BLK], Relu,
                bias=bv[:, 0:1],
            )
            nc.vector.tensor_scalar(
                h1[:, 1, :, 0:BLK], ps[1][:, :, 0:BLK],
                bv[:, 1:2], 0.0, op0=Alu.add, op1=Alu.max,
            )

            # Layer 2 writes back into the same psum tiles (bank cycling).
            for m in range(2):
                for j in range(2):
                    for k in range(2):
                        nc.tensor.matmul(
                            ps[m][:, j, 0:BLK],
                            w2_sb[k][:, m * 128 : (m + 1) * 128],
                            h1[:, k, j, 0:BLK],
                            start=(k == 0),
                            stop=(k == 1),
                        )

            scr0 = spool.tile([128, 2, 512], fp8, name="scr0", tag="scr0")
            nc.scalar.activation(
                scr0[:, :, 0:BLK],
                ps[0][:, :, 0:BLK],
                Relu,
                bias=bv[:, 2:3],
                accum_out=acc[0][:, pair : pair + 1],
            )
            scr1 = spool.tile([128, 2, 512], fp8, name="scr1", tag="scr1")
            nc.vector.tensor_scalar(
                scr1[:, :, 0:BLK],
                ps[1][:, :, 0:BLK],
                nb2[:],
                None,
                op0=Alu.max,
                op1=Alu.add,
                accum_out=acc[1][:, pair : pair + 1],
            )

        s_sb = cpool.tile([128, 2], f32, name="s_sb")
        for m in range(2):
            nc.vector.reduce_sum(s_sb[:, m : m + 1], acc[m][:], axis=X)
        nc.sync.dma_start(d_s, s_sb[:])

    nc.compile()
    return nc



def _build_v18(iters: int = 1):
    """v10 (strip-concurrent f16 L1 + f16 L2, DVE (a), ACT (b)x2) with fp8
    junk-scratch outputs on the two ACT (b) drains."""
    import concourse.mybir as mybir
    import concourse.tile as tile
    from concourse import bacc
    from contextlib import ExitStack

    dt = mybir.dt
    f32 = dt.float32
    f16 = dt.float16
    fp8 = dt.float8e4
    Relu = mybir.ActivationFunctionType.Relu
    Alu = mybir.AluOpType
    X = mybir.AxisListType.X
    R2 = R // 2

    nc = bacc.Bacc("TRN2", target_bir_lowering=False, debug=False,
                   enable_asserts=False, num_devices=1)

    d_xt = nc.dram_tensor("d_xt", [128, R2], f16, kind="ExternalInput").ap()
    d_w1 = nc.dram_tensor("d_w1", [128, H], f16, kind="ExternalInput").ap()
    d_w2 = nc.dram_tensor("d_w2", [H, H], f16, kind="ExternalInput").ap()
    d_b = nc.dram_tensor("d_b", [128, 4], f32, kind="ExternalInput").ap()
    d_s = nc.dram_tensor("d_s", [128, 2], f32, kind="ExternalOutput").ap()

    with tile.TileContext(nc) as tc, ExitStack() as ctx:
        cpool = ctx.enter_context(tc.tile_pool(name="cpool", bufs=1))
        xpool = ctx.enter_context(tc.tile_pool(name="xpool", bufs=3))
        hpool = ctx.enter_context(tc.tile_pool(name="hpool", bufs=2))
        spool = ctx.enter_context(tc.tile_pool(name="spool", bufs=1))
        ps1p = ctx.enter_context(tc.tile_pool(name="ps1p", bufs=1, space="PSUM"))
        ps2p = ctx.enter_context(tc.tile_pool(name="ps2p", bufs=1, space="PSUM"))

        w1_sb = cpool.tile([128, H], f16, name="w1_sb")
        nc.sync.dma_start(w1_sb[:], d_w1)
        w2_sb = []
        for k in range(2):
            tw = cpool.tile([128, H], f16, name=f"w2_sb{k}")
            nc.sync.dma_start(tw[:], d_w2[k * 128 : (k + 1) * 128, :])
            w2_sb.append(tw)
        bv = cpool.tile([128, 4], f32, name="bv")
        nc.sync.dma_start(bv[:], d_b)

        acc = [cpool.tile([128, NPAIR], f32, name=f"acc{m}") for m in range(2)]

        for pair in [p for _ in range(iters) for p in range(NPAIR)]:
            xt = xpool.tile([128, BLK], f16, name="xt", tag="xt")
            nc.sync.dma_start(xt[:], d_xt[:, pair * BLK : (pair + 1) * BLK])

            ps1 = [
                ps1p.tile([128, 2, 512], f32, name=f"ps1_{m}", tag=f"ps1_{m}")
                for m in range(2)
            ]
            ps2 = [
                ps2p.tile([128, 2, 512], f32, name=f"ps2_{m}", tag=f"ps2_{m}")
                for m in range(2)
            ]

            nc.tensor.matmul(ps1[0][:, 0, 0:BLK], w1_sb[0:64, 0:128],
                             xt[0:64, 0:BLK], start=True, stop=True)
            nc.tensor.matmul(ps1[0][:, 1, 0:BLK], w1_sb[64:128, 0:128],
                             xt[64:128, 0:BLK], start=True, stop=True)
            nc.tensor.matmul(ps1[1][:, 0, 0:BLK], w1_sb[0:64, 128:256],
                             xt[0:64, 0:BLK], start=True, stop=True)
            nc.tensor.matmul(ps1[1][:, 1, 0:BLK], w1_sb[64:128, 128:256],
                             xt[64:128, 0:BLK], start=True, stop=True)

            h1 = hpool.tile([128, 2, 2, 512], f16, name="h1", tag="h1")
            for m in range(2):
                nc.vector.tensor_scalar(
                    h1[:, m, :, 0:BLK], ps1[m][:, :, 0:BLK],
                    bv[:, m : m + 1], 0.0, op0=Alu.add, op1=Alu.max,
                )

            for j in range(2):
                for m in range(2):
                    for k in range(2):
                        nc.tensor.matmul(
                            ps2[m][:, j, 0:BLK],
                            w2_sb[k][:, m * 128 : (m + 1) * 128],
                            h1[:, k, j, 0:BLK],
                            start=(k == 0),
                            stop=(k == 1),
                        )

            for m in range(2):
                scr = spool.tile([128, 2, 512], fp8, name=f"scr{m}",
                                 tag=f"scr{m}")
                nc.scalar.activation(
                    scr[:, :, 0:BLK], ps2[m][:, :, 0:BLK], Relu,
                    bias=bv[:, 2 + m : 3 + m],
                    accum_out=acc[m][:, pair : pair + 1],
                )

        s_sb = cpool.tile([128, 2], f32, name="s_sb")
        for m in range(2):
            nc.vector.reduce_sum(s_sb[:, m : m + 1], acc[m][:], axis=X)
        nc.sync.dma_start(d_s, s_sb[:])

    nc.compile()
    return nc


def _build_base(mode: str, iters: int = 1, xbufs: int = 4, hbufs: int = 3):
    """The original staged baseline (f16 default): ones-row K=65 layer 1,
    f16 layer 2 in accumulation groups, DVE relu + 2 ACT relu+accum ops."""
    import concourse.mybir as mybir
    import concourse.tile as tile
    from concourse import bacc
    from contextlib import ExitStack

    dt = mybir.dt
    f32 = dt.float32
    split = mode == "f32r_split"
    mm_dt = {"f32r": dt.float32r, "f32r_split": dt.float32r, "f32": f32,
             "f16": dt.float16}[mode]

    nc = bacc.Bacc(
        "TRN2",
        target_bir_lowering=False,
        debug=False,
        enable_asserts=False,
        num_devices=1,
    )

    d_xt = nc.dram_tensor("d_xt", [IN + 1, R], mm_dt, kind="ExternalInput").ap()
    d_w1 = nc.dram_tensor("d_w1", [IN + 1, H], mm_dt, kind="ExternalInput").ap()
    d_w2 = nc.dram_tensor("d_w2", [H, H], mm_dt, kind="ExternalInput").ap()
    d_pb2 = nc.dram_tensor("d_pb2", [H], f32, kind="ExternalInput").ap()
    if split:
        d_w1l = nc.dram_tensor("d_w1l", [IN + 1, H], mm_dt, kind="ExternalInput").ap()
        d_w2l = nc.dram_tensor("d_w2l", [H, H], mm_dt, kind="ExternalInput").ap()
    d_s = nc.dram_tensor("d_s", [128, 2], f32, kind="ExternalOutput").ap()

    Relu = mybir.ActivationFunctionType.Relu
    X = mybir.AxisListType.X

    with tile.TileContext(nc) as tc, ExitStack() as ctx:
        cpool = ctx.enter_context(tc.tile_pool(name="cpool", bufs=1))
        xpool = ctx.enter_context(tc.tile_pool(name="xpool", bufs=xbufs))
        hpool = ctx.enter_context(tc.tile_pool(name="hpool", bufs=hbufs))
        spool = ctx.enter_context(tc.tile_pool(name="spool", bufs=2))
        ps1p = ctx.enter_context(tc.tile_pool(name="ps1p", bufs=2, space="PSUM"))
        ps2p = ctx.enter_context(tc.tile_pool(name="ps2p", bufs=2, space="PSUM"))

        w1_sb = cpool.tile([IN + 1, H], mm_dt, name="w1_sb")
        nc.sync.dma_start(w1_sb[:], d_w1)
        w2_sb = []
        for k in range(2):
            t = cpool.tile([128, H], mm_dt, name=f"w2_sb{k}")
            nc.sync.dma_start(t[:], d_w2[k * 128 : (k + 1) * 128, :])
            w2_sb.append(t)
        if split:
            w1l_sb = cpool.tile([IN + 1, H], mm_dt, name="w1l_sb")
            nc.sync.dma_start(w1l_sb[:], d_w1l)
            w2l_sb = []
            for k in range(2):
                t = cpool.tile([128, H], mm_dt, name=f"w2l_sb{k}")
                nc.sync.dma_start(t[:], d_w2l[k * 128 : (k + 1) * 128, :])
                w2l_sb.append(t)
        pb2_sb = cpool.tile([128, 2], f32, name="pb2_sb")
        nc.sync.dma_start(pb2_sb[:], d_pb2.rearrange("(m p) -> p m", p=128))

        acc = cpool.tile([128, 2, NBLK], f32, name="acc")

        for b in [b for _ in range(iters) for b in range(NBLK)]:
            xt = xpool.tile([IN + 1, BLK], mm_dt, name="xt", tag="xt")
            nc.sync.dma_start(xt[:], d_xt[:, b * BLK : (b + 1) * BLK])
            xr = xt[:]

            ps1 = ps1p.tile([128, 2, 512], f32, name="ps1", tag="ps1")
            for m in range(2):
                ms = slice(m * 128, (m + 1) * 128)
                nc.tensor.matmul(
                    ps1[:, m, 0:BLK], w1_sb[:, ms], xr,
                    start=True, stop=not split,
                )
                if split:
                    nc.tensor.matmul(
                        ps1[:, m, 0:BLK], w1l_sb[:, ms], xr,
                        start=False, stop=True,
                    )

            h1 = hpool.tile([128, 2, BLK], mm_dt, name="h1", tag="h1")
            nc.vector.tensor_scalar_max(h1[:], ps1[:, :, 0:BLK], 0.0)

            ps2 = ps2p.tile([128, 2, 512], f32, name="ps2", tag="ps2")
            for m in range(2):
                ms = slice(m * 128, (m + 1) * 128)
                mms = []
                for k in range(2):
                    mms.append((w2_sb[k][:, ms], h1[:, k, :]))
                    if split:
                        mms.append((w2l_sb[k][:, ms], h1[:, k, :]))
                for i, (lw, rr) in enumerate(mms):
                    nc.tensor.matmul(
                        ps2[:, m, 0:BLK], lw, rr,
                        start=(i == 0), stop=(i == len(mms) - 1),
                    )

            scr0 = spool.tile([128, BLK], f32, name="scr0", tag="scr0")
            nc.scalar.activation(
                scr0[:], ps2[:, 0, 0:BLK], Relu,
                bias=pb2_sb[:, 0:1],
                accum_out=acc[:, 0, b : b + 1],
            )
            scr1 = spool.tile([128, BLK], f32, name="scr1", tag="scr1")
            nc.scalar.activation(
                scr1[:], ps2[:, 1, 0:BLK], Relu,
                bias=pb2_sb[:, 1:2],
                accum_out=acc[:, 1, b : b + 1],
            )

        s_sb = cpool.tile([128, 2], f32, name="s_sb")
        nc.vector.reduce_sum(s_sb[:], acc[:], axis=X)
        nc.sync.dma_start(d_s, s_sb[:])

    nc.compile()
    return nc


def _hi_lo(w: np.ndarray):
    import ml_dtypes

    hi = np.asarray(w, dtype=ml_dtypes.bfloat16).astype(np.float32)
    lo = (w - hi).astype(np.float32)
    return hi, lo


def _diffuse_quant(W: np.ndarray, qdt) -> np.ndarray:
    """Error-diffusion quantization down the contraction axis: keeps
    per-column cumulative quantization error near zero so the (positive-mean)
    h1 stream doesn't see a systematic bias."""
    Wq = np.empty(W.shape, np.float32)
    carry = np.zeros(W.shape[1], np.float32)
    for k in range(W.shape[0]):
        t = W[k] + carry
        q = t.astype(qdt).astype(np.float32)
        carry = t - q
        Wq[k] = q
    return Wq


def _prep_in_maps(inputs: dict, mode: str):
    import ml_dtypes

    x = np.asarray(inputs["x"], dtype=np.float32)
    pw1 = np.asarray(inputs["pw1"], dtype=np.float32)
    pb1 = np.asarray(inputs["pb1"], dtype=np.float32)
    pw2 = np.asarray(inputs["pw2"], dtype=np.float32)
    pb2 = np.asarray(inputs["pb2"], dtype=np.float32)

    if mode in ("f16", "f32r", "f32r_split", "f32"):
        split = mode == "f32r_split"
        w1_aug = np.concatenate([pw1, pb1[None, :]], axis=0)  # [65, H]
        if split:
            w1h, w1l = _hi_lo(w1_aug)
            w2h, w2l = _hi_lo(pw2)
        else:
            w1h, w2h = w1_aug, pw2
        mm_np = np.float16 if mode == "f16" else np.float32
        w1h = w1h.astype(mm_np)
        w2h = w2h.astype(mm_np)
        in_maps = []
        for c in range(N_CORES):
            xt = np.empty((IN + 1, R), mm_np)
            xt[:IN] = x[c * R : (c + 1) * R].T.astype(mm_np)
            xt[IN] = 1.0
            m = {"d_xt": xt, "d_w1": w1h, "d_w2": w2h, "d_pb2": pb2}
            if split:
                m["d_w1l"] = w1l
                m["d_w2l"] = w2l
            in_maps.append(m)
        return in_maps

    if mode in ("v12", "v13"):
        import ml_dtypes

        # Pair channels by sorted b1 so both h-halves of a partition share
        # one (pair-mean) bias and the layer-1 drain is a single ACT op.
        order = np.argsort(pb1)
        pairs = order.reshape(128, 2)  # pairs[p] = channels (h0, h1) of p
        colmap = pairs.T.reshape(256)  # colmap[m*128+p] = orig channel
        bmean = 0.5 * (pb1[pairs[:, 0]] + pb1[pairs[:, 1]])  # [128]
        w1perm = pw1[:, colmap].astype(np.float16)  # [64, 256]
        w1d = np.concatenate([w1perm, w1perm], axis=0)  # [128, 256]
        W2perm = pw2[colmap, :]  # rows follow the h1 channel permutation
        W2q = _diffuse_quant(W2perm, ml_dtypes.float8_e4m3)
        w2 = np.ascontiguousarray(
            W2q.reshape(2, 128, H).transpose(1, 0, 2)
        ).astype(ml_dtypes.float8_e4m3)  # [k, pair, m]
        b = np.stack(
            [bmean, np.zeros(128, np.float32), pb2[0:128], pb2[128:256]],
            axis=1,
        ).astype(np.float32)
        x8 = np.asarray(inputs["x"], dtype=np.float32)
        in_maps = []
        for c in range(N_CORES):
            xc = x8[c * R : (c + 1) * R].T.astype(np.float16)  # [64, R]
            xr = xc.reshape(IN, NPAIR, 2, BLK)
            xt = np.concatenate(
                [
                    np.ascontiguousarray(xr[:, :, 0, :]).reshape(IN, R // 2),
                    np.ascontiguousarray(xr[:, :, 1, :]).reshape(IN, R // 2),
                ],
                axis=0,
            )  # [128, R//2]
            in_maps.append({"d_xt": xt, "d_w1": w1d, "d_w2": w2, "d_b": b})
        return in_maps

    fp8 = mode in ("v3fp8", "v4", "v5", "v6", "v7", "v8", "v16", "v17", "v19", "v21")
    if fp8:
        w2q = _diffuse_quant(pw2, ml_dtypes.float8_e4m3)
        w2 = np.ascontiguousarray(
            w2q.reshape(2, 128, H).transpose(1, 0, 2)
        ).astype(ml_dtypes.float8_e4m3)  # [k, pair, m]
    else:
        w2 = pw2.astype(np.float16)

    pw1h = pw1.astype(np.float16)
    common: dict
    if mode == "v4":
        b1m = np.zeros((128, 128), np.float16)
        b1m[64] = pb1[0:128].astype(np.float16)
        b1m[96] = pb1[128:256].astype(np.float16)
        b2m = np.stack([pb2[0:128], pb2[128:256]], axis=1).astype(np.float32)
        common = {"d_w1": pw1h, "d_w2": w2, "d_b1": b1m, "d_b2": b2m}
    elif mode == "v6":
        w1a = np.concatenate([pw1h, pb1[None, :].astype(np.float16)], axis=0)
        b2m = np.stack([pb2[0:128], pb2[128:256]], axis=1).astype(np.float32)
        common = {"d_w1": w1a, "d_w2": w2, "d_b2": b2m}
    elif mode in ("v7", "v8", "v9", "v10", "v11", "v16", "v17", "v18", "v19", "v20", "v21"):
        w1d = np.concatenate([pw1h, pw1h], axis=0)  # [128, 256]
        b = np.stack(
            [pb1[0:128], pb1[128:256], pb2[0:128], pb2[128:256]], axis=1
        ).astype(np.float32)
        common = {"d_w1": w1d, "d_w2": w2, "d_b": b}
    else:
        b = np.stack(
            [pb1[0:128], pb1[128:256], pb2[0:128], pb2[128:256]], axis=1
        ).astype(np.float32)  # [128, 4]
        common = {"d_w1": pw1h, "d_w2": w2, "d_b": b}

    in_maps = []
    for c in range(N_CORES):
        xc = x[c * R : (c + 1) * R].T.astype(np.float16)  # [64, R]
        if mode == "v6":
            xt = np.empty((IN + 1, R), np.float16)
            xt[:IN] = xc
            xt[IN] = 1.0
        elif mode in ("v7", "v8", "v10", "v11", "v16", "v17", "v18", "v19", "v20", "v21"):
            # interleave the pair's two 500-row blocks across partition
            # halves: [0:64] = even blocks, [64:128] = odd blocks.
            xr = xc.reshape(IN, NPAIR, 2, BLK)
            xt = np.concatenate(
                [
                    np.ascontiguousarray(xr[:, :, 0, :]).reshape(IN, R // 2),
                    np.ascontiguousarray(xr[:, :, 1, :]).reshape(IN, R // 2),
                ],
                axis=0,
            )  # [128, R//2]
        else:
            xt = np.ascontiguousarray(xc)
        in_maps.append({"d_xt": xt, **common})
    return in_maps


def _host_tail(S: np.ndarray, inputs: dict) -> np.ndarray:
    f = np.float64

    def g(name):
        return np.asarray(inputs[name], dtype=f)

    phi_sum = S @ g("pw3") + N * g("pb3")
    r = np.maximum(phi_sum @ g("rw1") + g("rb1"), 0.0)
    r = np.maximum(r @ g("rw2") + g("rb2"), 0.0)
    r = r @ g("rw3") + g("rb3")
    v = np.concatenate([r, g("x_static")])
    v = np.maximum(v @ g("w1") + g("b1"), 0.0)
    v = np.maximum(v @ g("w2") + g("b2"), 0.0)
    return (v @ g("w3") + g("b3")).astype(np.float32)


def _run(inputs: dict, trace: bool = False, mode: str | None = None):
    from concourse.bass_utils import run_bass_kernel_spmd

    mode = mode or MODE
    nc = _prog_cache.get(mode)
    if nc is None:
        nc = _build(mode)
        _prog_cache[mode] = nc

    if trace:
        try:
            import antenv.axon_hooks  # noqa: F401
        except ImportError:
            trace = False

    in_maps = _prep_in_maps(inputs, mode)
    res = run_bass_kernel_spmd(
        nc,
        in_maps,
        core_ids=list(range(N_CORES)),
        trace=trace,
    )

    S = np.zeros(H, np.float64)
    for rmap in res.results:
        s = rmap["d_s"].astype(np.float64)  # [128, 2]; channel = m*128 + p
        S += s.T.reshape(H)
    if mode in ("v4", "v5", "v6", "v7", "v8", "v11", "v12", "v13", "v16", "v17", "v19", "v21"):
        # the DVE path for h2-half1 accumulates sum(max(ps2, -b2)); the
        # + b2 * row-count shift is exact and lands here.
        S[128:256] += N * np.asarray(inputs["pb2"], np.float64)[128:256]
    out = _host_tail(S, inputs)
    return out, res


def kernel(**inputs) -> np.ndarray:
    out, _ = _run(inputs)
    return out

